# revision 1
# baseline (speedup 1.0000x reference)
"""AttnBlock (GroupNorm -> single-head self-attention -> residual) on 8 TRN2 cores.

Sharding: B=4 batch elements x 2 query-token halves = 8 cores (SPMD, no
collectives).  Each core receives the full (rolled) channel-major batch
element x^T [C=256, HW=4096], computes GroupNorm + k/v for all 4096
tokens, and q/scores/attention/out-proj for its 2048-token half.  Odd
cores get x rolled by -2048 tokens; attention is permutation-invariant
over keys, so their first 2048 tokens are the original tokens 2048:4096.

Layout is channel-major throughout (tokens on the free axis), which makes
every matmul transpose-free:
  hs^T = GN(x^T)                          [C, N]
  q^T = Wq^T.T @ hs^T  (lhsT=Wq^T)        [C, NQ]
  k^T likewise                            [C, N]
  v   = hs^T.T @ Wv^T  (lhsT=hs^T)        [N, C]   (row-major)
  S^T = k^T.T @ q^T    (lhsT=k^T)         [N, NQ]  (keys on partitions)
  P^T = exp(S^T/16)    (bf16)             softmax numerator, no max-sub
  Z   = ones.T @ P^T   (M=1 matmul)       [1, NQ]  denominators
  o^T = v.T @ P^T      (lhsT=v)           [C, NQ]; scaled by 1/Z
  out^T = Wo^T.T @ o^T + bo               [C, NQ]
  final = (x^T + out^T) / sqrt(2)
"""

import numpy as np
import ml_dtypes

import concourse.bass as bass
import concourse.tile as tile
from concourse import bacc, mybir
from concourse.bass_utils import run_bass_kernel_spmd

dt = mybir.dt
F32, F32R, BF16 = dt.float32, dt.float32r, dt.bfloat16
AF = mybir.ActivationFunctionType
ALU = mybir.AluOpType

P = 128          # partitions
C = 256          # channels
N = 4096         # tokens per batch element (64*64)
NQ = 2048        # query tokens per core
NSTRIP = 256     # query-token strip width
NS = NQ // NSTRIP  # 8 strips
MT = N // P      # 32 key m-tiles
GS = 8           # channels per group (256 / 32 groups)
EPS = 1e-6
ISCALE = 1.0 / 16.0      # attention scale c**-0.5
RS2 = float(2.0 ** -0.5)  # output residual scale

_prog_cache = {}


def _build_nc():
    nc = bacc.Bacc("TRN2", target_bir_lowering=False, debug=False, num_devices=8)

    def inp(name, shape, d=F32):
        return nc.dram_tensor(name, shape, d, kind="ExternalInput").ap()

    xt_d = inp("xt", [2, P, N])            # [c_half, c_in, n]
    wq_d = inp("wqT", [2, P, C])           # [ci_half, ci_in, c_out] = Wq.T
    wk_d = inp("wkT", [2, P, C])
    wv_d = inp("wvT", [2, P, C])
    wo_d = inp("woT", [2, P, C])
    bq_d = inp("bqp", [P, 2])              # [c_out_in, c_out_half]
    bk_d = inp("bkp", [P, 2])
    bos_d = inp("bosp", [P, 2])            # bo * 2^-0.5, packed
    bv_d = inp("bv", [1, C])
    gnw_d = inp("gnw", [P, 2])
    gnb_d = inp("gnb", [P, 2])
    amat_d = inp("amat", [P, P])           # block-diag 8x8 of 1/8
    ones1_d = inp("ones1", [1, P])
    onesm_d = inp("onesm", [P, 1], BF16)
    sel4_d = inp("sel4", [P, 1])           # 1.0 at partitions 0/32/64/96
    out_d = nc.dram_tensor("out", [2, P, NQ], F32, kind="ExternalOutput").ap()

    with tile.TileContext(nc) as tc:
        with (
            tc.tile_pool(name="singles", bufs=1) as singles,
            tc.tile_pool(name="xpool", bufs=1) as xpool,
            tc.tile_pool(name="hsfin", bufs=1) as hsfin,
            tc.tile_pool(name="qk", bufs=1) as qk,
            tc.tile_pool(name="vpool", bufs=1) as vpool,
            tc.tile_pool(name="espool", bufs=2) as espool,
            tc.tile_pool(name="opool", bufs=1) as opool,
            tc.tile_pool(name="small", bufs=2) as small,
            tc.tile_pool(name="zf", bufs=2) as zfpool,
            tc.tile_pool(name="ps", bufs=2, space="PSUM") as ps,
            tc.tile_pool(name="po", bufs=2, space="PSUM") as po,
            tc.tile_pool(name="pm", bufs=1, space="PSUM") as pm,
            tc.tile_pool(name="pz", bufs=1, space="PSUM") as pz,
        ):
            # ---- x load first: keep the HWDGE queues free of weight
            # traffic so GroupNorm stats start as soon as chunks land ----
# ---- x load (chunked; bn_stats pipelined behind each chunk) ----
            xt0 = xpool.tile([P, N], F32, tag="xt0")
            xt1 = xpool.tile([P, N], F32, tag="xt1")
            xts = (xt0, xt1)
            _dmae = [nc.sync, nc.scalar]
            for t in range(2):
                for h in range(4):
                    _dmae[h % 2].dma_start(
                        xts[t][:, h * 1024:(h + 1) * 1024],
                        xt_d[t, :, h * 1024:(h + 1) * 1024])

            # ---- constants / weights ----
            wq = singles.tile([P, 2, C], F32R)
            for _ko in range(2):
                nc.gpsimd.dma_start(wq[:, _ko, :], wq_d[_ko].bitcast(F32R))
            wk = singles.tile([P, 2, C], F32R)
            for _ko in range(2):
                nc.gpsimd.dma_start(wk[:, _ko, :], wk_d[_ko].bitcast(F32R))
            wv = singles.tile([P, 2, C], F32R)
            for _ko in range(2):
                nc.gpsimd.dma_start(wv[:, _ko, :], wv_d[_ko].bitcast(F32R))
            wo = singles.tile([P, 2, C], F32R)
            for _ko in range(2):
                nc.gpsimd.dma_start(wo[:, _ko, :], wo_d[_ko].bitcast(F32R))
            bq = singles.tile([P, 2], F32)
            nc.gpsimd.dma_start(bq[:], bq_d)
            bk = singles.tile([P, 2], F32)
            nc.gpsimd.dma_start(bk[:], bk_d)
            bos = singles.tile([P, 2], F32)
            nc.gpsimd.dma_start(bos[:], bos_d)
            gnw = singles.tile([P, 2], F32)
            nc.gpsimd.dma_start(gnw[:], gnw_d)
            gnb = singles.tile([P, 2], F32)
            nc.gpsimd.dma_start(gnb[:], gnb_d)
            amat = singles.tile([P, P], F32R)
            nc.gpsimd.dma_start(amat[:], amat_d.bitcast(F32R))
            ones1 = singles.tile([1, P], F32R)
            nc.gpsimd.dma_start(ones1[:], ones1_d.bitcast(F32R))
            onesm = singles.tile([P, 1], BF16)
            nc.gpsimd.dma_start(onesm[:], onesm_d)
            sel4 = singles.tile([P, 1], F32R)
            nc.gpsimd.dma_start(sel4[:], sel4_d.bitcast(F32R))
            # bv broadcast to all partitions (stride-0 partition DMA)
            bvrep = singles.tile([P, C], F32)
            bv_b = bass.AP(tensor=bv_d.tensor, offset=bv_d.offset,
                           ap=[[0, P], bv_d.ap[1]])
            nc.gpsimd.dma_start(out=bvrep[:], in_=bv_b)
            epsap = singles.tile([P, 1], F32)
            nc.vector.memset(epsap[:], EPS)

            # ---- GroupNorm (channel-major; stats per channel then 8-chan groups) ----
            hs = hsfin.tile([P, 2, N], F32R, tag="hsfin")
            for t in range(2):
                st = small.tile([P, 8, 6], F32, tag="gnst")
                xre = xts[t][:, :].rearrange("p (s f) -> p s f", f=512)
                for sg in range(8):
                    nc.vector.bn_stats(st[:, sg, :], xre[:, sg, :])
                mv = small.tile([P, 2], F32, tag="gnmv")
                nc.vector.bn_aggr(mv[:], st[:])
                # stats2 = [mu, E[x^2]] per channel, rounded to f32r for the matmul
                musq = small.tile([P, 1], F32, tag="gnmusq")
                nc.vector.tensor_mul(musq[:], mv[:, 0:1], mv[:, 0:1])
                stats2 = small.tile([P, 2], F32R, tag="gnst2")
                nc.vector.tensor_copy(stats2[:, 0:1], mv[:, 0:1])
                nc.vector.tensor_add(stats2[:, 1:2], mv[:, 1:2], musq[:])
                # group-aggregate (mean over 8 channels) and broadcast back
                gp = pm.tile([P, 512], F32, tag="pm")
                nc.tensor.matmul(gp[:, 0:2], amat[:], stats2[:], start=True, stop=True)
                gs = small.tile([P, 2], F32, tag="gnagg")
                nc.vector.tensor_copy(gs[:], gp[:, 0:2])
                gvar = small.tile([P, 1], F32, tag="gnvar")
                gmusq = small.tile([P, 1], F32, tag="gnmusq2")
                nc.vector.tensor_mul(gmusq[:], gs[:, 0:1], gs[:, 0:1])
                nc.vector.tensor_tensor(gvar[:], gs[:, 1:2], gmusq[:], ALU.subtract)
                # rstd = exp(-0.5 * ln(var + eps))  (same ACT table set as softmax exp)
                lnv = small.tile([P, 1], F32, tag="gnln")
                nc.scalar.activation(lnv[:], gvar[:], AF.Ln, bias=epsap[:], scale=1.0)
                rstd = small.tile([P, 1], F32, tag="gnrstd")
                nc.scalar.activation(rstd[:], lnv[:], AF.Exp, bias=0.0, scale=-0.5)
                alpha = small.tile([P, 1], F32, tag="gnalpha")
                nc.vector.tensor_mul(alpha[:], rstd[:], gnw[:, t:t + 1])
                atmp = small.tile([P, 1], F32, tag="gnatmp")
                nc.vector.tensor_mul(atmp[:], gs[:, 0:1], alpha[:])
                beta = small.tile([P, 1], F32, tag="gnbeta")
                nc.vector.tensor_tensor(beta[:], gnb[:, t:t + 1], atmp[:], ALU.subtract)
                for hh in range(2):
                    nc.vector.tensor_scalar(hs[:, t, hh * 2048:(hh + 1) * 2048],
                                            xts[t][:, hh * 2048:(hh + 1) * 2048],
                                            alpha[:], beta[:], ALU.mult, ALU.add)

            # ---- projections ----
            qT = qk.tile([P, 2, NQ], F32R, tag="qT")
            kT = qk.tile([P, 2, N], F32R, tag="kT")
            for (wt, bt, dst, nblk) in ((wq, bq, qT, NQ // 256), (wk, bk, kT, N // 256)):
                for ch in range(2):
                    for j in range(nblk // 2):
                        sp = ps.tile([P, 4, NSTRIP], F32, tag="ps")
                        for i in range(2):
                            b = 2 * j + i
                            for ko in range(2):
                                nc.tensor.matmul(
                                    sp[:, i, :],
                                    wt[:, ko, ch * P:(ch + 1) * P],
                                    hs[:, ko, b * 256:(b + 1) * 256],
                                    start=(ko == 0), stop=(ko == 1))
                        nc.vector.tensor_scalar(
                            dst[:, ch, 2 * j * 256:(2 * j + 2) * 256],
                            sp[:, 0:2, :].rearrange("p a b -> p (a b)"),
                            bt[:, ch:ch + 1], None, ALU.add)
            v = vpool.tile([P, MT, C], BF16)
            for m in range(MT):
                if m % 2 == 0:
                    vpt = pm.tile([P, 512], F32, tag="pm", name=f"vp{m}")
                    vp = vpt[:, 0:C]
                else:
                    vpt = po.tile([P, 2, NSTRIP], F32, tag="po", name=f"vp{m}")
                    vp = vpt[:, 0, :]
                for ko in range(2):
                    nc.tensor.matmul(vp, hs[:, ko, m * P:(m + 1) * P],
                                     wv[:, ko, :], start=(ko == 0), stop=(ko == 1))
                nc.vector.tensor_add(v[:, m, :], vp, bvrep[:])

            # ---- attention strips ----
            final = hsfin.tile([P, 2, NQ], F32, tag="hsfin")
            for s in range(NS):
                ns = slice(s * NSTRIP, (s + 1) * NSTRIP)
                es = espool.tile([P, MT, NSTRIP], BF16, tag="es")
                for j in range(MT // 4):
                    sp = ps.tile([P, 4, NSTRIP], F32, tag="ps")
                    for i in range(4):
                        m = 4 * j + i
                        for ko in range(2):
                            nc.tensor.matmul(sp[:, i, :], kT[:, ko, m * P:(m + 1) * P],
                                             qT[:, ko, ns], start=(ko == 0), stop=(ko == 1))
                    nc.scalar.activation(es[:, 4 * j:4 * j + 4, :], sp[:],
                                         AF.Exp, bias=0.0, scale=ISCALE)
                # softmax denominators: Z = ones.T @ P^T, 4 col-packed M=1 chains
                zp = pz.tile([P, NSTRIP], F32, tag="pz")
                for j in range(MT // 4):
                    for c in range(4):
                        nc.tensor.matmul(zp[32 * c:32 * c + 1, :], onesm[:],
                                         es[:, 4 * j + c, :],
                                         start=(j == 0), stop=(j == MT // 4 - 1),
                                         tile_position=(0, 32 * c))
                # attn @ v
                op = po.tile([P, 2, NSTRIP], F32, tag="po")
                for ch in range(2):
                    for m in range(MT):
                        nc.tensor.matmul(op[:, ch, :], v[:, m, ch * P:(ch + 1) * P],
                                         es[:, m, :], start=(m == 0), stop=(m == MT - 1))
                # Z = sel4.T @ zsb picks+sums the 4 packed rows (others hold garbage)
                zsb = small.tile([P, NSTRIP], F32R, tag="zsb")
                nc.vector.tensor_copy(zsb[:], zp[:])
                zqt = pm.tile([P, 512], F32, tag="pm", name=f"zq{s}")
                nc.tensor.matmul(zqt[0:1, 0:NSTRIP], sel4[:], zsb[:],
                                 start=True, stop=True)
                # 1/Z on DVE (avoids ACT table-set thrash), broadcast via K=1 matmul
                rz = small.tile([1, NSTRIP], F32R, tag="rz")
                with nc.allow_low_precision(reason="f32r rounding of 1/Z"):
                    nc.vector.reciprocal(rz[:], zqt[0:1, 0:NSTRIP])
                rp = pm.tile([P, 512], F32, tag="pm")
                nc.tensor.matmul(rp[:, 0:NSTRIP], ones1[:], rz[:], start=True, stop=True)
                rzs = small.tile([P, NSTRIP], F32, tag="rzs")
                nc.vector.tensor_copy(rzs[:], rp[:, 0:NSTRIP])
                o = opool.tile([P, 2, NQ], F32R, tag="o")
                for ch in range(2):
                    nc.vector.tensor_mul(o[:, ch, ns], op[:, ch, :], rzs[:])
                # out projection + bias + residual + 2^-0.5 (psum from po — free
                # here, and keeps pm's single slot off the strip critical path)
                op2 = po.tile([P, 2, NSTRIP], F32, tag="po", name=f"op2_{s}")
                for ch in range(2):
                    for ko in range(2):
                        nc.tensor.matmul(op2[:, ch, :],
                                         wo[:, ko, ch * P:(ch + 1) * P],
                                         o[:, ko, ns], start=(ko == 0), stop=(ko == 1))
                z2 = zfpool.tile([P, 2, NSTRIP], F32, tag="zf")
                for ch in range(2):
                    nc.scalar.activation(z2[:, ch, :], op2[:, ch, :],
                                         AF.Identity, bias=bos[:, ch:ch + 1], scale=RS2)
                for t in range(2):
                    nc.vector.scalar_tensor_tensor(
                        out=final[:, t, ns], in0=xts[t][:, ns], scalar=RS2,
                        in1=z2[:, t, :], op0=ALU.mult, op1=ALU.add)
                    nc.sync.dma_start(out_d[t, :, ns], final[:, t, ns])

    nc.finalize()
    return nc


def _get_nc():
    if "nc" not in _prog_cache:
        _prog_cache["nc"] = _build_nc()
    return _prog_cache["nc"]


def _make_in_maps(x, gn_weight, gn_bias, Wq, bq, Wk, bk, Wv, bv, Wo, bo):
    x = np.asarray(x, dtype=np.float32)
    f32 = lambda a: np.ascontiguousarray(np.asarray(a, dtype=np.float32))

    def packT(b_vec):  # [256] -> [128, 2] (c_out_in, c_out_half)
        return np.ascontiguousarray(f32(b_vec).reshape(2, P).T)

    amat = np.zeros((P, P), np.float32)
    for g in range(P // GS):
        amat[g * GS:(g + 1) * GS, g * GS:(g + 1) * GS] = 1.0 / GS
    sel4 = np.zeros((P, 1), np.float32)
    sel4[[0, 32, 64, 96], 0] = 1.0

    common = {
        "wqT": f32(np.asarray(Wq).T).reshape(2, P, C),
        "wkT": f32(np.asarray(Wk).T).reshape(2, P, C),
        "wvT": f32(np.asarray(Wv).T).reshape(2, P, C),
        "woT": f32(np.asarray(Wo).T).reshape(2, P, C),
        "bqp": packT(bq),
        "bkp": packT(bk),
        "bosp": packT(np.asarray(bo, dtype=np.float32) * RS2),
        "bv": f32(bv).reshape(1, C),
        "gnw": packT(gn_weight),
        "gnb": packT(gn_bias),
        "amat": amat,
        "ones1": np.ones((1, P), np.float32),
        "onesm": np.ones((P, 1), ml_dtypes.bfloat16),
        "sel4": sel4,
    }

    in_maps = []
    for core in range(8):
        b, half = core // 2, core % 2
        xt = x[b].reshape(C, N)
        if half:
            xt = np.roll(xt, -NQ, axis=1)
        in_maps.append({"xt": np.ascontiguousarray(xt).reshape(2, P, N), **common})
    return in_maps


def _assemble(results, B):
    out = np.empty((B, C, N), np.float32)
    for core in range(2 * B):
        b, half = core // 2, core % 2
        out[b, :, half * NQ:(half + 1) * NQ] = results[core]["out"].reshape(C, NQ)
    return out.reshape(B, C, 64, 64)


def kernel(x, gn_weight, gn_bias, Wq, bq, Wk, bk, Wv, bv, Wo, bo):
    x = np.asarray(x, dtype=np.float32)
    in_maps = _make_in_maps(x, gn_weight, gn_bias, Wq, bq, Wk, bk, Wv, bv, Wo, bo)
    nc = _get_nc()
    res = run_bass_kernel_spmd(nc, in_maps, list(range(8)))
    return _assemble(res.results, x.shape[0])



# revision 24
# speedup vs baseline: 1.5604x; 1.5604x over previous
"""AttnBlock (GroupNorm -> single-head self-attention -> residual) on 8 TRN2 cores.

Sharding: B=4 batch elements x 2 query-token halves = 8 cores (SPMD, no
collectives).  Each core receives the full (rolled) channel-major batch
element x^T [C=256, HW=4096] in bf16, computes GroupNorm stats + k/v for
all 4096 tokens, and q/scores/attention/out-proj for its 2048-token half.
Odd cores get x rolled by -2048 tokens; attention is permutation-invariant
over keys, so their first 2048 tokens are the original tokens 2048:4096.

Matmul strategy: GroupNorm is folded into the projections (alpha into the
bf16 weights, beta into per-channel biases via tiny K=1 matmuls), so the
q/k/v projections read x^T directly.  The attention matmuls (scores,
softmax-denominator chain, attn@v, out-proj) run in fp8e4m3 with
perf_mode=DoubleRow, which packs the full K=256 contraction into a single
PE pass at 2 MACs/cell/cycle.  exp uses a -3 logit bias so the fp8
softmax numerator stays within e4m3 range (the factor cancels in the
normalization).  1/Z is computed after transposing Z onto partitions
(a [1,256] single-lane reciprocal is ~16x slower than a [128,2] one),
and the normalization is applied after the out-projection (a per-query
column scale commutes with the channel-space projection).  bv and bo
fold into one final bias; x + out is scaled by 2^-0.5 at the end.
"""

import numpy as np
import ml_dtypes

import concourse.bass as bass
import concourse.tile as tile
from concourse import bacc, mybir
from concourse.bass_utils import run_bass_kernel_spmd

dt = mybir.dt
F32, F32R, BF16, F8 = dt.float32, dt.float32r, dt.bfloat16, dt.float8e4
AF = mybir.ActivationFunctionType
ALU = mybir.AluOpType
DR = mybir.MatmulPerfMode.DoubleRow

P = 128          # partitions
C = 256          # channels
N = 4096         # tokens per batch element (64*64)
NQ = 2048        # query tokens per core
NSTRIP = 256     # query-token strip width
NS = NQ // NSTRIP  # 8 strips
MT = N // P      # 32 key m-tiles
GS = 8           # channels per group (256 / 32 groups)
EPS = 1e-6
ISCALE = 1.0 / 16.0       # attention scale c**-0.5
EBIAS = -3.0              # exp logit bias; cancels in softmax normalization
RS2 = float(2.0 ** -0.5)  # output residual scale

_prog_cache = {}


def _build_nc():
    nc = bacc.Bacc("TRN2", target_bir_lowering=False, debug=False, num_devices=8)

    def inp(name, shape, d=F32):
        return nc.dram_tensor(name, shape, d, kind="ExternalInput").ap()

    xt_d = inp("xt", [2, P, N], BF16)      # [ci_half, ci_in, n]
    wq_d = inp("wqT", [P, 2, C], BF16)     # [ci_in, ci_half, c_out] = Wq.T pairs
    wk_d = inp("wkT", [P, 2, C], BF16)
    wv_d = inp("wvT", [P, 2, C], BF16)
    wo_d = inp("woT", [P, 2, C], BF16)
    bq_d = inp("bqp", [P, 2])              # [c_out_in, c_out_half]
    bk_d = inp("bkp", [P, 2])
    bv_d = inp("bvp", [P, 2])
    bo_d = inp("bop", [P, 2])
    gnw_d = inp("gnw", [P, 2])
    gnb_d = inp("gnb", [P, 2])
    amat_d = inp("amat", [P, P])           # block-diag 8x8 of 1/8
    ones2_d = inp("ones2", [1, P], BF16)   # value RS2 (folds residual scale into rz)
    ones8_d = inp("ones8", [P, 2, 16], F8)  # 1.0; 16-padded for DR weight AP
    idm_d = inp("idm", [P, P], BF16)       # identity (partition<->free moves via PE)
    out_d = nc.dram_tensor("out", [2, P, NQ], F32, kind="ExternalOutput").ap()

    with tile.TileContext(nc) as tc:
        with (
            tc.tile_pool(name="singles", bufs=1) as singles,
            tc.tile_pool(name="xpool", bufs=1) as xpool,
            tc.tile_pool(name="qk", bufs=1) as qk,
            tc.tile_pool(name="vpool", bufs=1) as vpool,
            tc.tile_pool(name="espool", bufs=2) as espool,
            tc.tile_pool(name="opool", bufs=2) as opool,
            tc.tile_pool(name="small", bufs=2) as small,
            tc.tile_pool(name="zf", bufs=2) as zfpool,
            tc.tile_pool(name="ps", bufs=2, space="PSUM") as ps,      # 2x2 banks
            tc.tile_pool(name="po", bufs=2, space="PSUM") as po,      # 2x1 banks
            tc.tile_pool(name="pz", bufs=1, space="PSUM") as pz,      # 1 bank
            tc.tile_pool(name="px", bufs=1, space="PSUM") as px,      # 1 bank
        ):
            # ---- x load first (chunked; bn_stats pipelined behind each chunk) ----
            xt0 = xpool.tile([P, N], BF16, tag="xt0")
            xt1 = xpool.tile([P, N], BF16, tag="xt1")
            xts = (xt0, xt1)
            _dmae = [nc.sync, nc.scalar]
            for t in range(2):
                for h in range(4):
                    _dmae[h % 2].dma_start(
                        xts[t][:, h * 1024:(h + 1) * 1024],
                        xt_d[t, :, h * 1024:(h + 1) * 1024])

            # ---- weights / constants ----
            wq = singles.tile([P, 2, C], BF16)
            nc.gpsimd.dma_start(wq[:], wq_d)
            wk = singles.tile([P, 2, C], BF16)
            nc.gpsimd.dma_start(wk[:], wk_d)
            wv = singles.tile([P, 2, C], BF16)
            nc.gpsimd.dma_start(wv[:], wv_d)
            wo = singles.tile([P, 2, C], BF16)
            nc.gpsimd.dma_start(wo[:], wo_d)
            bq = singles.tile([P, 2], F32)
            nc.gpsimd.dma_start(bq[:], bq_d)
            bk = singles.tile([P, 2], F32)
            nc.gpsimd.dma_start(bk[:], bk_d)
            bvp = singles.tile([P, 2], F32)
            nc.gpsimd.dma_start(bvp[:], bv_d)
            bop = singles.tile([P, 2], F32)
            nc.gpsimd.dma_start(bop[:], bo_d)
            gnw = singles.tile([P, 2], F32)
            nc.gpsimd.dma_start(gnw[:], gnw_d)
            gnb = singles.tile([P, 2], F32)
            nc.gpsimd.dma_start(gnb[:], gnb_d)
            amat = singles.tile([P, P], F32R)
            nc.gpsimd.dma_start(amat[:], amat_d.bitcast(F32R))
            ones2 = singles.tile([1, P], BF16)
            nc.gpsimd.dma_start(ones2[:], ones2_d)
            ones8 = singles.tile([P, 2, 16], F8)
            nc.gpsimd.dma_start(ones8[:], ones8_d)
            idm = singles.tile([P, P], BF16)
            nc.gpsimd.dma_start(idm[:], idm_d)
            epsap = singles.tile([P, 1], F32)
            nc.vector.memset(epsap[:], EPS)
            ebias = singles.tile([P, 1], F32)
            nc.vector.memset(ebias[:], EBIAS)

            # ---- GroupNorm stats (channel-major; per channel then 8-chan groups) ----
            alf = small.tile([P, 2], F32, tag="gnalf")   # alpha per ci half
            bet = small.tile([P, 2], F32, tag="gnbet")   # beta per ci half
            for t in range(2):
                st = small.tile([P, 8, 6], F32, tag="gnst")
                xre = xts[t][:, :].rearrange("p (s f) -> p s f", f=512)
                for sg in range(8):
                    nc.vector.bn_stats(st[:, sg, :], xre[:, sg, :])
                mv = small.tile([P, 2], F32, tag="gnmv")
                nc.vector.bn_aggr(mv[:], st[:])
                # stats2 = [mu, E[x^2]] per channel, f32r for the group matmul
                musq = small.tile([P, 1], F32, tag="gnmusq")
                nc.vector.tensor_mul(musq[:], mv[:, 0:1], mv[:, 0:1])
                stats2 = small.tile([P, 2], F32R, tag="gnst2")
                nc.vector.tensor_copy(stats2[:, 0:1], mv[:, 0:1])
                nc.vector.tensor_add(stats2[:, 1:2], mv[:, 1:2], musq[:])
                # group-aggregate (mean over 8 channels)
                gp = px.tile([P, 2], F32, tag="px", name=f"gp{t}_")
                nc.tensor.matmul(gp[:], amat[:], stats2[:], start=True, stop=True)
                gs = small.tile([P, 2], F32, tag="gnagg")
                nc.vector.tensor_copy(gs[:], gp[:])
                gvar = small.tile([P, 1], F32, tag="gnvar")
                gmusq = small.tile([P, 1], F32, tag="gnmusq2")
                nc.vector.tensor_mul(gmusq[:], gs[:, 0:1], gs[:, 0:1])
                nc.vector.tensor_tensor(gvar[:], gs[:, 1:2], gmusq[:], ALU.subtract)
                # rstd = exp(-0.5 * ln(var + eps))  (same ACT table set as softmax exp)
                lnv = small.tile([P, 1], F32, tag="gnln")
                nc.scalar.activation(lnv[:], gvar[:], AF.Ln, bias=epsap[:], scale=1.0)
                rstd = small.tile([P, 1], F32, tag="gnrstd")
                nc.scalar.activation(rstd[:], lnv[:], AF.Exp, bias=0.0, scale=-0.5)
                nc.vector.tensor_mul(alf[:, t:t + 1], rstd[:], gnw[:, t:t + 1])
                atmp = small.tile([P, 1], F32, tag="gnatmp")
                nc.vector.tensor_mul(atmp[:], gs[:, 0:1], alf[:, t:t + 1])
                nc.vector.tensor_tensor(bet[:, t:t + 1], gnb[:, t:t + 1], atmp[:],
                                        ALU.subtract)

            # ---- bias plumbing: fold GN beta (and bv, bo) into projection biases ----
            bet_bf = small.tile([P, 2], BF16, tag="betbf")
            nc.vector.tensor_copy(bet_bf[:], bet[:])
            # W @ beta columns for q, k, v (K=1-wide matmuls on raw bf16 weights)
            wbeta = pz.tile([P, 2, 3], F32, tag="pz", name="wbeta")
            for wi, wt in enumerate((wq, wk, wv)):
                for ch in range(2):
                    for ko in range(2):
                        nc.tensor.matmul(wbeta[:, ch, wi:wi + 1],
                                         wt[:, ko, ch * P:(ch + 1) * P],
                                         bet_bf[:, ko:ko + 1],
                                         start=(ko == 0), stop=(ko == 1))
            qbias = small.tile([P, 2], F32, tag="qbias")
            nc.vector.tensor_add(qbias[:], wbeta[:, :, 0], bq[:])
            kbias = small.tile([P, 2], F32, tag="kbias")
            nc.vector.tensor_add(kbias[:], wbeta[:, :, 1], bk[:])
            vbias_bf = small.tile([P, 2], BF16, tag="vbiasbf")
            nc.vector.tensor_add(vbias_bf[:], wbeta[:, :, 2], bvp[:])
            # bfin = (Wo @ (Wv@beta + bv) + bo) * RS2
            obias = pz.tile([P, 2], F32, tag="pz", name="obias")
            for ch in range(2):
                for ko in range(2):
                    nc.tensor.matmul(obias[:, ch:ch + 1],
                                     wo[:, ko, ch * P:(ch + 1) * P],
                                     vbias_bf[:, ko:ko + 1],
                                     start=(ko == 0), stop=(ko == 1))
            bfin = small.tile([P, 2], F32, tag="bfin")
            for ch in range(2):
                nc.vector.tensor_scalar(bfin[:, ch:ch + 1], obias[:, ch:ch + 1],
                                        bop[:, ch:ch + 1], RS2, ALU.add, ALU.mult)

            # ---- fold GN alpha into q/k/v weights; cast wo to fp8 pairs ----
            wqf = singles.tile([P, 2, C], BF16, name="wqf")
            wkf = singles.tile([P, 2, C], BF16, name="wkf")
            wvf = singles.tile([P, 2, C], BF16, name="wvf")
            for wt, wf in ((wq, wqf), (wk, wkf), (wv, wvf)):
                for t in range(2):
                    nc.vector.tensor_scalar(wf[:, t, :], wt[:, t, :],
                                            alf[:, t:t + 1], None, ALU.mult)
            wo8 = singles.tile([P, 2, C], F8, name="wo8")
            nc.vector.tensor_copy(wo8[:], wo[:])

            # ---- projections (bf16 weights x bf16 x; outputs cast to fp8) ----
            qT = qk.tile([P, 2, NQ], F8, tag="qT")
            kT = qk.tile([P, 2, N], F8, tag="kT")
            v8 = vpool.tile([P, MT, C], F8)
            # interleave q/k/v blocks so ACT/DVE casts chase the PE
            for blk in range(8):
                # k block: 512 tokens
                kps = ps.tile([P, 2, 512], F32, tag="ps", name=f"kps{blk}")
                for ch in range(2):
                    for ko in range(2):
                        nc.tensor.matmul(kps[:, ch, :],
                                         wkf[:, ko, ch * P:(ch + 1) * P],
                                         xts[ko][:, blk * 512:(blk + 1) * 512],
                                         start=(ko == 0), stop=(ko == 1))
                for ch in range(2):
                    nc.vector.tensor_scalar(kT[:, ch, blk * 512:(blk + 1) * 512],
                                            kps[:, ch, :], kbias[:, ch:ch + 1],
                                            None, ALU.add)
                if blk < 4:
                    # q block: 512 tokens (first NQ only)
                    qps = ps.tile([P, 2, 512], F32, tag="ps", name=f"qps{blk}")
                    for ch in range(2):
                        for ko in range(2):
                            nc.tensor.matmul(qps[:, ch, :],
                                             wqf[:, ko, ch * P:(ch + 1) * P],
                                             xts[ko][:, blk * 512:(blk + 1) * 512],
                                             start=(ko == 0), stop=(ko == 1))
                    for ch in range(2):
                        nc.scalar.activation(qT[:, ch, blk * 512:(blk + 1) * 512],
                                             qps[:, ch, :], AF.Identity,
                                             bias=qbias[:, ch:ch + 1], scale=1.0)
                # v block: 4 m-tiles (512 tokens)
                vps = ps.tile([P, 2, 512], F32, tag="ps", name=f"vps{blk}")
                vpv = vps[:, :, :].rearrange("p a b -> p (a b)").rearrange(
                    "p (i c) -> p i c", c=C)
                for i in range(4):
                    m = 4 * blk + i
                    for ko in range(2):
                        nc.tensor.matmul(vpv[:, i, :],
                                         xts[ko][:, m * P:(m + 1) * P],
                                         wvf[:, ko, :],
                                         start=(ko == 0), stop=(ko == 1))
                nc.scalar.activation(v8[:, 4 * blk:4 * blk + 4, :], vpv[:],
                                     AF.Identity, bias=0.0, scale=1.0)

            # ---- attention strips (fp8 DoubleRow) ----
            for s in range(NS):
                ns = slice(s * NSTRIP, (s + 1) * NSTRIP)
                es = espool.tile([P, MT, NSTRIP], F8, tag="es")
                # scores S^T(m-tile) = (k pair).T @ (q pair); exp in 4-m groups
                for g in range(MT // 4):
                    sp = ps.tile([P, 4, NSTRIP], F32, tag="ps", name=f"sp{s}_{g}")
                    for i in range(4):
                        m = 4 * g + i
                        nc.tensor.matmul(sp[:, i, :],
                                         kT[:, 0:2, m * P:(m + 1) * P],
                                         qT[:, 0:2, ns],
                                         start=True, stop=True, perf_mode=DR)
                    nc.scalar.activation(es[:, 4 * g:4 * g + 4, :], sp[:],
                                         AF.Exp, bias=ebias[:], scale=ISCALE)
                # softmax denominators: single DR chain over 16 m-pairs
                zp = pz.tile([1, NSTRIP], F32, tag="pz", name=f"zp{s}")
                for j in range(MT // 2):
                    nc.tensor.matmul(zp[:], ones8[:, 0:2, 0:1],
                                     es[:, 2 * j:2 * j + 2, :],
                                     start=(j == 0), stop=(j == MT // 2 - 1),
                                     perf_mode=DR)
                # attn @ v: DR over m-pairs, accumulated per channel half
                op = po.tile([P, 2, NSTRIP], F32, tag="po", name=f"op{s}")
                for ch in range(2):
                    for j in range(MT // 2):
                        nc.tensor.matmul(op[:, ch, :],
                                         v8[:, 2 * j:2 * j + 2, ch * P:(ch + 1) * P],
                                         es[:, 2 * j:2 * j + 2, :],
                                         start=(j == 0), stop=(j == MT // 2 - 1),
                                         perf_mode=DR)
                # 1/Z: move Z onto partitions (K=1 matmul), reciprocal there,
                # move back (identity matmul), broadcast (RS2 folded into ones2)
                zs = small.tile([1, NSTRIP], BF16, tag="zs")
                nc.vector.tensor_copy(zs[:], zp[:])
                zqt = pz.tile([P, 2], F32, tag="pz", name=f"zqt{s}")
                for h in range(2):
                    nc.tensor.matmul(zqt[:, h:h + 1], zs[0:1, h * P:(h + 1) * P],
                                     idm[0:1, 0:1], start=True, stop=True)
                rzt = small.tile([P, 2], BF16, tag="rzt")
                with nc.allow_low_precision(reason="bf16 rounding of 1/Z"):
                    nc.vector.reciprocal(rzt[:], zqt[:])
                rzc = pz.tile([1, NSTRIP], F32, tag="pz", name=f"rzc{s}")
                for h in range(2):
                    nc.tensor.matmul(rzc[0:1, h * P:(h + 1) * P], rzt[:, h:h + 1],
                                     idm[:], start=True, stop=True)
                rzs = small.tile([1, NSTRIP], BF16, tag="rzs")
                nc.vector.tensor_copy(rzs[:], rzc[:])
                rp2 = px.tile([P, NSTRIP], F32, tag="px", name=f"rp2{s}")
                for h in range(2):
                    nc.tensor.matmul(rp2[:, h * P:(h + 1) * P], ones2[:],
                                     rzs[0:1, h * P:(h + 1) * P],
                                     start=True, stop=True)
                rp2s = small.tile([P, NSTRIP], BF16, tag="rp2s")
                nc.vector.tensor_copy(rp2s[:], rp2[:])
                # out projection on unnormalized o (fp8 DR), then per-query scale
                o8 = opool.tile([P, 2, NSTRIP], F8, tag="o8")
                nc.vector.tensor_copy(o8[:], op[:])
                op2 = po.tile([P, 2, NSTRIP], F32, tag="po", name=f"op2{s}")
                for ch in range(2):
                    nc.tensor.matmul(op2[:, ch, :], wo8[:, 0:2, ch * P:(ch + 1) * P],
                                     o8[:, 0:2, :], start=True, stop=True,
                                     perf_mode=DR)
                t1 = zfpool.tile([P, 2, NSTRIP], BF16, tag="t1")
                for ch in range(2):
                    nc.vector.tensor_mul(t1[:, ch, :], op2[:, ch, :], rp2s[:])
                z2 = zfpool.tile([P, 2, NSTRIP], BF16, tag="zf")
                for ch in range(2):
                    nc.scalar.activation(z2[:, ch, :], t1[:, ch, :],
                                         AF.Identity, bias=bfin[:, ch:ch + 1],
                                         scale=1.0)
                final = zfpool.tile([P, 2, NSTRIP], F32, tag="fin")
                for t in range(2):
                    nc.vector.scalar_tensor_tensor(
                        out=final[:, t, :], in0=xts[t][:, ns], scalar=RS2,
                        in1=z2[:, t, :], op0=ALU.mult, op1=ALU.add)
                    nc.sync.dma_start(out_d[t, :, ns], final[:, t, :])

    nc.finalize()
    return nc


def _get_nc():
    if "nc" not in _prog_cache:
        _prog_cache["nc"] = _build_nc()
    return _prog_cache["nc"]


def _make_in_maps(x, gn_weight, gn_bias, Wq, bq, Wk, bk, Wv, bv, Wo, bo):
    x = np.asarray(x, dtype=np.float32)
    f32 = lambda a: np.ascontiguousarray(np.asarray(a, dtype=np.float32))

    def packT(b_vec):  # [256] -> [128, 2] (c_out_in, c_out_half)
        return np.ascontiguousarray(f32(b_vec).reshape(2, P).T)

    def packW(W):  # [C, C] -> [P, 2, C] bf16 pairs: w[ci_in, ko, co] = W[co, ko*128+ci]
        wT = f32(np.asarray(W).T).reshape(2, P, C)  # [ko, ci_in, co]
        return np.ascontiguousarray(wT.transpose(1, 0, 2)).astype(ml_dtypes.bfloat16)

    amat = np.zeros((P, P), np.float32)
    for g in range(P // GS):
        amat[g * GS:(g + 1) * GS, g * GS:(g + 1) * GS] = 1.0 / GS

    common = {
        "wqT": packW(Wq),
        "wkT": packW(Wk),
        "wvT": packW(Wv),
        "woT": packW(Wo),
        "bqp": packT(bq),
        "bkp": packT(bk),
        "bvp": packT(bv),
        "bop": packT(bo),
        "gnw": packT(gn_weight),
        "gnb": packT(gn_bias),
        "amat": amat,
        "ones2": np.full((1, P), RS2, ml_dtypes.bfloat16),
        "ones8": np.ones((P, 2, 16), ml_dtypes.float8_e4m3),
        "idm": np.eye(P, dtype=ml_dtypes.bfloat16),
    }

    in_maps = []
    for core in range(8):
        b, half = core // 2, core % 2
        xt = x[b].reshape(C, N)
        if half:
            xt = np.roll(xt, -NQ, axis=1)
        xt = np.ascontiguousarray(xt).reshape(2, P, N).astype(ml_dtypes.bfloat16)
        in_maps.append({"xt": xt, **common})
    return in_maps


def _assemble(results, B):
    out = np.empty((B, C, N), np.float32)
    for core in range(2 * B):
        b, half = core // 2, core % 2
        out[b, :, half * NQ:(half + 1) * NQ] = results[core]["out"].reshape(C, NQ)
    return out.reshape(B, C, 64, 64)


def kernel(x, gn_weight, gn_bias, Wq, bq, Wk, bk, Wv, bv, Wo, bo):
    x = np.asarray(x, dtype=np.float32)
    in_maps = _make_in_maps(x, gn_weight, gn_bias, Wq, bq, Wk, bk, Wv, bv, Wo, bo)
    nc = _get_nc()
    res = run_bass_kernel_spmd(nc, in_maps, list(range(8)))
    return _assemble(res.results, x.shape[0])


# revision 27
# speedup vs baseline: 1.7686x; 1.1334x over previous
"""AttnBlock (GroupNorm -> single-head self-attention -> residual) on 8 TRN2 cores.

Sharding: B=4 batch elements x 2 query-token halves = 8 cores (SPMD, no
collectives).  Each core receives the full (rolled) channel-major batch
element x^T [C=256, HW=4096] in bf16, computes GroupNorm stats + k/v for
all 4096 tokens, and q/scores/attention/out-proj for its 2048-token half.
Odd cores get x rolled by -2048 tokens; attention is permutation-invariant
over keys, so their first 2048 tokens are the original tokens 2048:4096.

Matmul strategy: GroupNorm is folded into the projections (alpha into the
bf16 weights, beta into per-channel biases via tiny K=1 matmuls), so the
q/k/v projections read x^T directly.  The attention matmuls (scores,
softmax-denominator chain, attn@v, out-proj) run in fp8e4m3 with
perf_mode=DoubleRow, which packs the full K=256 contraction into a single
PE pass at 2 MACs/cell/cycle.  exp uses a -3 logit bias so the fp8
softmax numerator stays within e4m3 range (the factor cancels in the
normalization).  1/Z is computed after transposing Z onto partitions
(a [1,256] single-lane reciprocal is ~16x slower than a [128,2] one),
and the normalization is applied after the out-projection (a per-query
column scale commutes with the channel-space projection).  bv and bo
fold into one final bias; x + out is scaled by 2^-0.5 at the end.
"""

import numpy as np
import ml_dtypes

import concourse.bass as bass
import concourse.tile as tile
from concourse import bacc, mybir
from concourse.bass_utils import run_bass_kernel_spmd

dt = mybir.dt
F32, F32R, BF16, F8 = dt.float32, dt.float32r, dt.bfloat16, dt.float8e4
AF = mybir.ActivationFunctionType
ALU = mybir.AluOpType
DR = mybir.MatmulPerfMode.DoubleRow

P = 128          # partitions
C = 256          # channels
N = 4096         # tokens per batch element (64*64)
NQ = 2048        # query tokens per core
NSTRIP = 512     # query-token strip width
NS = NQ // NSTRIP  # 8 strips
MT = N // P      # 32 key m-tiles
GS = 8           # channels per group (256 / 32 groups)
EPS = 1e-6
ISCALE = 1.0 / 16.0       # attention scale c**-0.5
EBIAS = -3.0              # exp logit bias; cancels in softmax normalization
RS2 = float(2.0 ** -0.5)  # output residual scale

_prog_cache = {}


def _build_nc():
    nc = bacc.Bacc("TRN2", target_bir_lowering=False, debug=False, num_devices=8)

    def inp(name, shape, d=F32):
        return nc.dram_tensor(name, shape, d, kind="ExternalInput").ap()

    xt_d = inp("xt", [2, P, N], BF16)      # [ci_half, ci_in, n]
    wq_d = inp("wqT", [P, 2, C], BF16)     # [ci_in, ci_half, c_out] = Wq.T pairs
    wk_d = inp("wkT", [P, 2, C], BF16)
    wv_d = inp("wvT", [P, 2, C], BF16)
    wo_d = inp("woT", [P, 2, C], BF16)
    bq_d = inp("bqp", [P, 2])              # [c_out_in, c_out_half]
    bk_d = inp("bkp", [P, 2])
    bv_d = inp("bvp", [P, 2])
    bo_d = inp("bop", [P, 2])
    gnw_d = inp("gnw", [P, 2])
    gnb_d = inp("gnb", [P, 2])
    amat_d = inp("amat", [P, P])           # block-diag 8x8 of 1/8
    ones2_d = inp("ones2", [1, P], BF16)   # value RS2 (folds residual scale into rz)
    ones8_d = inp("ones8", [P, 2, 16], F8)  # 1.0; 16-padded for DR weight AP
    idm_d = inp("idm", [P, P], BF16)       # identity (partition<->free moves via PE)
    out_d = nc.dram_tensor("out", [2, P, NQ], F32, kind="ExternalOutput").ap()

    with tile.TileContext(nc) as tc:
        with (
            tc.tile_pool(name="singles", bufs=1) as singles,
            tc.tile_pool(name="xpool", bufs=1) as xpool,
            tc.tile_pool(name="qk", bufs=1) as qk,
            tc.tile_pool(name="vpool", bufs=1) as vpool,
            tc.tile_pool(name="espool", bufs=2) as espool,
            tc.tile_pool(name="opool", bufs=2) as opool,
            tc.tile_pool(name="small", bufs=2) as small,
            tc.tile_pool(name="zf", bufs=2) as zfpool,
            tc.tile_pool(name="ps", bufs=2, space="PSUM") as ps,      # 2x2 banks
            tc.tile_pool(name="po", bufs=1, space="PSUM") as po,      # 2 banks
            tc.tile_pool(name="pz", bufs=1, space="PSUM") as pz,      # 1 bank
        ):
            # ---- x load first (chunked; bn_stats pipelined behind each chunk) ----
            xt0 = xpool.tile([P, N], BF16, tag="xt0")
            xt1 = xpool.tile([P, N], BF16, tag="xt1")
            xts = (xt0, xt1)
            _dmae = [nc.sync, nc.scalar]
            for t in range(2):
                for h in range(4):
                    _dmae[h % 2].dma_start(
                        xts[t][:, h * 1024:(h + 1) * 1024],
                        xt_d[t, :, h * 1024:(h + 1) * 1024])

            # ---- weights / constants ----
            wq = singles.tile([P, 2, C], BF16)
            nc.gpsimd.dma_start(wq[:], wq_d)
            wk = singles.tile([P, 2, C], BF16)
            nc.gpsimd.dma_start(wk[:], wk_d)
            wv = singles.tile([P, 2, C], BF16)
            nc.gpsimd.dma_start(wv[:], wv_d)
            wo = singles.tile([P, 2, C], BF16)
            nc.gpsimd.dma_start(wo[:], wo_d)
            bq = singles.tile([P, 2], F32)
            nc.gpsimd.dma_start(bq[:], bq_d)
            bk = singles.tile([P, 2], F32)
            nc.gpsimd.dma_start(bk[:], bk_d)
            bvp = singles.tile([P, 2], F32)
            nc.gpsimd.dma_start(bvp[:], bv_d)
            bop = singles.tile([P, 2], F32)
            nc.gpsimd.dma_start(bop[:], bo_d)
            gnw = singles.tile([P, 2], F32)
            nc.gpsimd.dma_start(gnw[:], gnw_d)
            gnb = singles.tile([P, 2], F32)
            nc.gpsimd.dma_start(gnb[:], gnb_d)
            amat = singles.tile([P, P], F32R)
            nc.gpsimd.dma_start(amat[:], amat_d.bitcast(F32R))
            ones2 = singles.tile([1, P], BF16)
            nc.gpsimd.dma_start(ones2[:], ones2_d)
            ones8 = singles.tile([P, 2, 16], F8)
            nc.gpsimd.dma_start(ones8[:], ones8_d)
            idm = singles.tile([P, P], BF16)
            nc.gpsimd.dma_start(idm[:], idm_d)
            epsap = singles.tile([P, 1], F32)
            nc.vector.memset(epsap[:], EPS)
            ebias = singles.tile([P, 1], F32)
            nc.vector.memset(ebias[:], EBIAS)

            # ---- GroupNorm stats (channel-major; per channel then 8-chan groups) ----
            alf = small.tile([P, 2], F32, tag="gnalf")   # alpha per ci half
            bet = small.tile([P, 2], F32, tag="gnbet")   # beta per ci half
            for t in range(2):
                st = small.tile([P, 8, 6], F32, tag="gnst")
                xre = xts[t][:, :].rearrange("p (s f) -> p s f", f=512)
                for sg in range(8):
                    nc.vector.bn_stats(st[:, sg, :], xre[:, sg, :])
                mv = small.tile([P, 2], F32, tag="gnmv")
                nc.vector.bn_aggr(mv[:], st[:])
                # stats2 = [mu, E[x^2]] per channel, f32r for the group matmul
                musq = small.tile([P, 1], F32, tag="gnmusq")
                nc.vector.tensor_mul(musq[:], mv[:, 0:1], mv[:, 0:1])
                stats2 = small.tile([P, 2], F32R, tag="gnst2")
                nc.vector.tensor_copy(stats2[:, 0:1], mv[:, 0:1])
                nc.vector.tensor_add(stats2[:, 1:2], mv[:, 1:2], musq[:])
                # group-aggregate (mean over 8 channels)
                gp = pz.tile([P, 2], F32, tag="pz", name=f"gp{t}_")
                nc.tensor.matmul(gp[:], amat[:], stats2[:], start=True, stop=True)
                gs = small.tile([P, 2], F32, tag="gnagg")
                nc.vector.tensor_copy(gs[:], gp[:])
                gvar = small.tile([P, 1], F32, tag="gnvar")
                gmusq = small.tile([P, 1], F32, tag="gnmusq2")
                nc.vector.tensor_mul(gmusq[:], gs[:, 0:1], gs[:, 0:1])
                nc.vector.tensor_tensor(gvar[:], gs[:, 1:2], gmusq[:], ALU.subtract)
                # rstd = exp(-0.5 * ln(var + eps))  (same ACT table set as softmax exp)
                lnv = small.tile([P, 1], F32, tag="gnln")
                nc.scalar.activation(lnv[:], gvar[:], AF.Ln, bias=epsap[:], scale=1.0)
                rstd = small.tile([P, 1], F32, tag="gnrstd")
                nc.scalar.activation(rstd[:], lnv[:], AF.Exp, bias=0.0, scale=-0.5)
                nc.vector.tensor_mul(alf[:, t:t + 1], rstd[:], gnw[:, t:t + 1])
                atmp = small.tile([P, 1], F32, tag="gnatmp")
                nc.vector.tensor_mul(atmp[:], gs[:, 0:1], alf[:, t:t + 1])
                nc.vector.tensor_tensor(bet[:, t:t + 1], gnb[:, t:t + 1], atmp[:],
                                        ALU.subtract)

            # ---- bias plumbing: fold GN beta (and bv, bo) into projection biases ----
            bet_bf = small.tile([P, 2], BF16, tag="betbf")
            nc.vector.tensor_copy(bet_bf[:], bet[:])
            # W @ beta columns for q, k, v (K=1-wide matmuls on raw bf16 weights)
            wbeta = pz.tile([P, 2, 3], F32, tag="pz", name="wbeta")
            for wi, wt in enumerate((wq, wk, wv)):
                for ch in range(2):
                    for ko in range(2):
                        nc.tensor.matmul(wbeta[:, ch, wi:wi + 1],
                                         wt[:, ko, ch * P:(ch + 1) * P],
                                         bet_bf[:, ko:ko + 1],
                                         start=(ko == 0), stop=(ko == 1))
            qbias = small.tile([P, 2], F32, tag="qbias")
            nc.vector.tensor_add(qbias[:], wbeta[:, :, 0], bq[:])
            kbias = small.tile([P, 2], F32, tag="kbias")
            nc.vector.tensor_add(kbias[:], wbeta[:, :, 1], bk[:])
            vbias_bf = small.tile([P, 2], BF16, tag="vbiasbf")
            nc.vector.tensor_add(vbias_bf[:], wbeta[:, :, 2], bvp[:])
            # bfin = (Wo @ (Wv@beta + bv) + bo) * RS2
            obias = pz.tile([P, 2], F32, tag="pz", name="obias")
            for ch in range(2):
                for ko in range(2):
                    nc.tensor.matmul(obias[:, ch:ch + 1],
                                     wo[:, ko, ch * P:(ch + 1) * P],
                                     vbias_bf[:, ko:ko + 1],
                                     start=(ko == 0), stop=(ko == 1))
            bfin = small.tile([P, 2], F32, tag="bfin")
            for ch in range(2):
                nc.vector.tensor_scalar(bfin[:, ch:ch + 1], obias[:, ch:ch + 1],
                                        bop[:, ch:ch + 1], RS2, ALU.add, ALU.mult)

            # ---- fold GN alpha into q/k/v weights; cast wo to fp8 pairs ----
            wqf = singles.tile([P, 2, C], BF16, name="wqf")
            wkf = singles.tile([P, 2, C], BF16, name="wkf")
            wvf = singles.tile([P, 2, C], BF16, name="wvf")
            for wt, wf in ((wq, wqf), (wk, wkf), (wv, wvf)):
                for t in range(2):
                    nc.vector.tensor_scalar(wf[:, t, :], wt[:, t, :],
                                            alf[:, t:t + 1], None, ALU.mult)
            wo8 = singles.tile([P, 2, C], F8, name="wo8")
            nc.vector.tensor_copy(wo8[:], wo[:])

            # ---- projections (bf16 weights x bf16 x; outputs cast to fp8) ----
            qT = qk.tile([P, 2, NQ], F8, tag="qT")
            kT = qk.tile([P, 2, N], F8, tag="kT")
            v8 = vpool.tile([P, MT, C], F8)
            # interleave q/k/v blocks so ACT/DVE casts chase the PE
            for blk in range(8):
                # k block: 512 tokens
                kps = ps.tile([P, 2, 512], F32, tag="ps", name=f"kps{blk}")
                for ch in range(2):
                    for ko in range(2):
                        nc.tensor.matmul(kps[:, ch, :],
                                         wkf[:, ko, ch * P:(ch + 1) * P],
                                         xts[ko][:, blk * 512:(blk + 1) * 512],
                                         start=(ko == 0), stop=(ko == 1))
                for ch in range(2):
                    nc.vector.tensor_scalar(kT[:, ch, blk * 512:(blk + 1) * 512],
                                            kps[:, ch, :], kbias[:, ch:ch + 1],
                                            None, ALU.add)
                if blk < 4:
                    # q block: 512 tokens (first NQ only)
                    qps = po.tile([P, 2, 512], F32, tag="po", name=f"qps{blk}")
                    for ch in range(2):
                        for ko in range(2):
                            nc.tensor.matmul(qps[:, ch, :],
                                             wqf[:, ko, ch * P:(ch + 1) * P],
                                             xts[ko][:, blk * 512:(blk + 1) * 512],
                                             start=(ko == 0), stop=(ko == 1))
                    for ch in range(2):
                        nc.scalar.activation(qT[:, ch, blk * 512:(blk + 1) * 512],
                                             qps[:, ch, :], AF.Identity,
                                             bias=qbias[:, ch:ch + 1], scale=1.0)
                # v block: 4 m-tiles (512 tokens)
                vps = ps.tile([P, 2, 512], F32, tag="ps", name=f"vps{blk}")
                vpv = vps[:, :, :].rearrange("p a b -> p (a b)").rearrange(
                    "p (i c) -> p i c", c=C)
                for i in range(4):
                    m = 4 * blk + i
                    for ko in range(2):
                        nc.tensor.matmul(vpv[:, i, :],
                                         xts[ko][:, m * P:(m + 1) * P],
                                         wvf[:, ko, :],
                                         start=(ko == 0), stop=(ko == 1))
                nc.scalar.activation(v8[:, 4 * blk:4 * blk + 4, :], vpv[:],
                                     AF.Identity, bias=0.0, scale=1.0)

            # ---- attention strips (fp8 DoubleRow) ----
            NH = NSTRIP // P  # 128-wide query groups per strip
            for s in range(NS):
                ns = slice(s * NSTRIP, (s + 1) * NSTRIP)
                es = espool.tile([P, MT, NSTRIP], F8, tag="es")
                # scores S^T(m-tile) = (k pair).T @ (q pair); exp in 2-m groups
                for g in range(MT // 2):
                    sp = ps.tile([P, 2, NSTRIP], F32, tag="ps", name=f"sp{s}_{g}")
                    for i in range(2):
                        m = 2 * g + i
                        nc.tensor.matmul(sp[:, i, :],
                                         kT[:, 0:2, m * P:(m + 1) * P],
                                         qT[:, 0:2, ns],
                                         start=True, stop=True, perf_mode=DR)
                    nc.scalar.activation(es[:, 2 * g:2 * g + 2, :], sp[:],
                                         AF.Exp, bias=ebias[:], scale=ISCALE)
                # softmax denominators: single DR chain over 16 m-pairs
                zp = pz.tile([1, NSTRIP], F32, tag="pz", name=f"zp{s}")
                for j in range(MT // 2):
                    nc.tensor.matmul(zp[:], ones8[:, 0:2, 0:1],
                                     es[:, 2 * j:2 * j + 2, :],
                                     start=(j == 0), stop=(j == MT // 2 - 1),
                                     perf_mode=DR)
                # attn @ v: DR over m-pairs, accumulated per channel half
                op = po.tile([P, 2, NSTRIP], F32, tag="po", name=f"op{s}")
                for ch in range(2):
                    for j in range(MT // 2):
                        nc.tensor.matmul(op[:, ch, :],
                                         v8[:, 2 * j:2 * j + 2, ch * P:(ch + 1) * P],
                                         es[:, 2 * j:2 * j + 2, :],
                                         start=(j == 0), stop=(j == MT // 2 - 1),
                                         perf_mode=DR)
                # 1/Z: move Z onto partitions (K=1 matmul), reciprocal there,
                # move back (identity matmul), broadcast (RS2 folded into ones2)
                zs = small.tile([1, NSTRIP], BF16, tag="zs")
                nc.vector.tensor_copy(zs[:], zp[:])
                zqt = pz.tile([P, NH], F32, tag="pz", name=f"zqt{s}")
                for h in range(NH):
                    nc.tensor.matmul(zqt[:, h:h + 1], zs[0:1, h * P:(h + 1) * P],
                                     idm[0:1, 0:1], start=True, stop=True)
                rzt = small.tile([P, NH], BF16, tag="rzt")
                with nc.allow_low_precision(reason="bf16 rounding of 1/Z"):
                    nc.vector.reciprocal(rzt[:], zqt[:])
                rzc = pz.tile([1, NSTRIP], F32, tag="pz", name=f"rzc{s}")
                for h in range(NH):
                    nc.tensor.matmul(rzc[0:1, h * P:(h + 1) * P], rzt[:, h:h + 1],
                                     idm[:], start=True, stop=True)
                rzs = small.tile([1, NSTRIP], BF16, tag="rzs")
                nc.vector.tensor_copy(rzs[:], rzc[:])
                rp2 = pz.tile([P, NSTRIP], F32, tag="pz", name=f"rp2{s}")
                for h in range(NH):
                    nc.tensor.matmul(rp2[:, h * P:(h + 1) * P], ones2[:],
                                     rzs[0:1, h * P:(h + 1) * P],
                                     start=True, stop=True)
                rp2s = small.tile([P, NSTRIP], BF16, tag="rp2s")
                nc.vector.tensor_copy(rp2s[:], rp2[:])
                # out projection on unnormalized o (fp8 DR), then per-query scale
                o8 = opool.tile([P, 2, NSTRIP], F8, tag="o8")
                nc.vector.tensor_copy(o8[:], op[:])
                op2 = ps.tile([P, 2, NSTRIP], F32, tag="ps", name=f"op2{s}")
                for ch in range(2):
                    nc.tensor.matmul(op2[:, ch, :], wo8[:, 0:2, ch * P:(ch + 1) * P],
                                     o8[:, 0:2, :], start=True, stop=True,
                                     perf_mode=DR)
                t1 = zfpool.tile([P, 2, NSTRIP], BF16, tag="t1")
                for ch in range(2):
                    nc.vector.tensor_mul(t1[:, ch, :], op2[:, ch, :], rp2s[:])
                z2 = zfpool.tile([P, 2, NSTRIP], BF16, tag="zf")
                for ch in range(2):
                    nc.scalar.activation(z2[:, ch, :], t1[:, ch, :],
                                         AF.Identity, bias=bfin[:, ch:ch + 1],
                                         scale=1.0)
                final = zfpool.tile([P, 2, NSTRIP], F32, tag="fin")
                for t in range(2):
                    nc.vector.scalar_tensor_tensor(
                        out=final[:, t, :], in0=xts[t][:, ns], scalar=RS2,
                        in1=z2[:, t, :], op0=ALU.mult, op1=ALU.add)
                    nc.sync.dma_start(out_d[t, :, ns], final[:, t, :])

    nc.finalize()
    return nc


def _get_nc():
    if "nc" not in _prog_cache:
        _prog_cache["nc"] = _build_nc()
    return _prog_cache["nc"]


def _make_in_maps(x, gn_weight, gn_bias, Wq, bq, Wk, bk, Wv, bv, Wo, bo):
    x = np.asarray(x, dtype=np.float32)
    f32 = lambda a: np.ascontiguousarray(np.asarray(a, dtype=np.float32))

    def packT(b_vec):  # [256] -> [128, 2] (c_out_in, c_out_half)
        return np.ascontiguousarray(f32(b_vec).reshape(2, P).T)

    def packW(W):  # [C, C] -> [P, 2, C] bf16 pairs: w[ci_in, ko, co] = W[co, ko*128+ci]
        wT = f32(np.asarray(W).T).reshape(2, P, C)  # [ko, ci_in, co]
        return np.ascontiguousarray(wT.transpose(1, 0, 2)).astype(ml_dtypes.bfloat16)

    amat = np.zeros((P, P), np.float32)
    for g in range(P // GS):
        amat[g * GS:(g + 1) * GS, g * GS:(g + 1) * GS] = 1.0 / GS

    common = {
        "wqT": packW(Wq),
        "wkT": packW(Wk),
        "wvT": packW(Wv),
        "woT": packW(Wo),
        "bqp": packT(bq),
        "bkp": packT(bk),
        "bvp": packT(bv),
        "bop": packT(bo),
        "gnw": packT(gn_weight),
        "gnb": packT(gn_bias),
        "amat": amat,
        "ones2": np.full((1, P), RS2, ml_dtypes.bfloat16),
        "ones8": np.ones((P, 2, 16), ml_dtypes.float8_e4m3),
        "idm": np.eye(P, dtype=ml_dtypes.bfloat16),
    }

    in_maps = []
    for core in range(8):
        b, half = core // 2, core % 2
        xt = x[b].reshape(C, N)
        if half:
            xt = np.roll(xt, -NQ, axis=1)
        xt = np.ascontiguousarray(xt).reshape(2, P, N).astype(ml_dtypes.bfloat16)
        in_maps.append({"xt": xt, **common})
    return in_maps


def _assemble(results, B):
    out = np.empty((B, C, N), np.float32)
    for core in range(2 * B):
        b, half = core // 2, core % 2
        out[b, :, half * NQ:(half + 1) * NQ] = results[core]["out"].reshape(C, NQ)
    return out.reshape(B, C, 64, 64)


def kernel(x, gn_weight, gn_bias, Wq, bq, Wk, bk, Wv, bv, Wo, bo):
    x = np.asarray(x, dtype=np.float32)
    in_maps = _make_in_maps(x, gn_weight, gn_bias, Wq, bq, Wk, bk, Wv, bv, Wo, bo)
    nc = _get_nc()
    res = run_bass_kernel_spmd(nc, in_maps, list(range(8)))
    return _assemble(res.results, x.shape[0])


# revision 32
# speedup vs baseline: 1.8470x; 1.0443x over previous
"""AttnBlock (GroupNorm -> single-head self-attention -> residual) on 8 TRN2 cores.

Sharding: B=4 batch elements x 2 query-token halves = 8 cores (SPMD, no
collectives).  Each core receives the full (rolled) channel-major batch
element x^T [C=256, HW=4096] in bf16, computes GroupNorm stats + k/v for
all 4096 tokens, and q/scores/attention/out-proj for its 2048-token half.
Odd cores get x rolled by -2048 tokens; attention is permutation-invariant
over keys, so their first 2048 tokens are the original tokens 2048:4096.

Matmul strategy: GroupNorm is folded into the projections (alpha into the
bf16 weights, beta into per-channel biases via tiny K=1 matmuls), so the
q/k/v projections read x^T directly.  The attention matmuls (scores,
softmax-denominator chain, attn@v, out-proj) run in fp8e4m3 with
perf_mode=DoubleRow, which packs the full K=256 contraction into a single
PE pass at 2 MACs/cell/cycle.  exp uses a -3 logit bias so the fp8
softmax numerator stays within e4m3 range (the factor cancels in the
normalization).  1/Z is computed after transposing Z onto partitions
(a [1,256] single-lane reciprocal is ~16x slower than a [128,2] one),
and the normalization is applied after the out-projection (a per-query
column scale commutes with the channel-space projection).  bv and bo
fold into one final bias; x + out is scaled by 2^-0.5 at the end.
"""

import numpy as np
import ml_dtypes

import concourse.bass as bass
import concourse.tile as tile
from concourse import bacc, mybir
from concourse.bass_utils import run_bass_kernel_spmd

dt = mybir.dt
F32, F32R, BF16, F8 = dt.float32, dt.float32r, dt.bfloat16, dt.float8e4
AF = mybir.ActivationFunctionType
ALU = mybir.AluOpType
DR = mybir.MatmulPerfMode.DoubleRow

P = 128          # partitions
C = 256          # channels
N = 4096         # tokens per batch element (64*64)
NQ = 2048        # query tokens per core
NSTRIP = 512     # query-token strip width
NS = NQ // NSTRIP  # 8 strips
MT = N // P      # 32 key m-tiles
GS = 8           # channels per group (256 / 32 groups)
EPS = 1e-6
ISCALE = 1.0 / 16.0       # attention scale c**-0.5
EBIAS = -3.0              # exp logit bias; cancels in softmax normalization
RS2 = float(2.0 ** -0.5)  # output residual scale

_prog_cache = {}


def _build_nc():
    nc = bacc.Bacc("TRN2", target_bir_lowering=False, debug=False, num_devices=8)

    def inp(name, shape, d=F32):
        return nc.dram_tensor(name, shape, d, kind="ExternalInput").ap()

    xt_d = inp("xt", [2, P, N], BF16)      # [ci_half, ci_in, n]
    wq_d = inp("wqT", [P, 2, C], BF16)     # [ci_in, ci_half, c_out] = Wq.T pairs
    wk_d = inp("wkT", [P, 2, C], BF16)
    wv_d = inp("wvT", [P, 2, C], BF16)
    wo_d = inp("woT", [P, 2, C], BF16)
    bq_d = inp("bqp", [P, 2])              # [c_out_in, c_out_half]
    bk_d = inp("bkp", [P, 2])
    bv_d = inp("bvp", [P, 2])
    bo_d = inp("bop", [P, 2])
    gnw_d = inp("gnw", [P, 2])
    gnb_d = inp("gnb", [P, 2])
    amat_d = inp("amat", [P, P])           # block-diag 8x8 of 1/8
    ones2_d = inp("ones2", [1, P], BF16)   # value RS2 (folds residual scale into rz)
    ones8_d = inp("ones8", [P, 2, 16], F8)  # 1.0; 16-padded for DR weight AP
    idm_d = inp("idm", [P, P], BF16)       # identity (partition<->free moves via PE)
    out_d = nc.dram_tensor("out", [2, P, NQ], F32, kind="ExternalOutput").ap()

    with tile.TileContext(nc) as tc:
        with (
            tc.tile_pool(name="singles", bufs=1) as singles,
            tc.tile_pool(name="xpool", bufs=1) as xpool,
            tc.tile_pool(name="qk", bufs=1) as qk,
            tc.tile_pool(name="vpool", bufs=1) as vpool,
            tc.tile_pool(name="espool", bufs=2) as espool,
            tc.tile_pool(name="opool", bufs=2) as opool,
            tc.tile_pool(name="small", bufs=2) as small,
            tc.tile_pool(name="zf", bufs=2) as zfpool,
            tc.tile_pool(name="ps", bufs=2, space="PSUM") as ps,      # 2x2 banks
            tc.tile_pool(name="po", bufs=1, space="PSUM") as po,      # 2 banks
            tc.tile_pool(name="pz", bufs=1, space="PSUM") as pz,      # 1 bank
        ):
            # ---- x load first (chunked; bn_stats pipelined behind each chunk) ----
            xt0 = xpool.tile([P, N], BF16, tag="xt0")
            xt1 = xpool.tile([P, N], BF16, tag="xt1")
            xts = (xt0, xt1)
            _dmae = [nc.sync, nc.scalar]
            for t in range(2):
                for h in range(4):
                    _dmae[h % 2].dma_start(
                        xts[t][:, h * 1024:(h + 1) * 1024],
                        xt_d[t, :, h * 1024:(h + 1) * 1024])

            # ---- weights / constants ----
            wq = singles.tile([P, 2, C], BF16)
            nc.gpsimd.dma_start(wq[:], wq_d)
            wk = singles.tile([P, 2, C], BF16)
            nc.gpsimd.dma_start(wk[:], wk_d)
            wv = singles.tile([P, 2, C], BF16)
            nc.gpsimd.dma_start(wv[:], wv_d)
            wo = singles.tile([P, 2, C], BF16)
            nc.gpsimd.dma_start(wo[:], wo_d)
            bq = singles.tile([P, 2], F32)
            nc.gpsimd.dma_start(bq[:], bq_d)
            bk = singles.tile([P, 2], F32)
            nc.gpsimd.dma_start(bk[:], bk_d)
            bvp = singles.tile([P, 2], F32)
            nc.gpsimd.dma_start(bvp[:], bv_d)
            bop = singles.tile([P, 2], F32)
            nc.gpsimd.dma_start(bop[:], bo_d)
            gnw = singles.tile([P, 2], F32)
            nc.gpsimd.dma_start(gnw[:], gnw_d)
            gnb = singles.tile([P, 2], F32)
            nc.gpsimd.dma_start(gnb[:], gnb_d)
            amat = singles.tile([P, P], F32R)
            nc.gpsimd.dma_start(amat[:], amat_d.bitcast(F32R))
            ones2 = singles.tile([1, P], BF16)
            nc.gpsimd.dma_start(ones2[:], ones2_d)
            ones8 = singles.tile([P, 2, 16], F8)
            nc.gpsimd.dma_start(ones8[:], ones8_d)
            idm = singles.tile([P, P], BF16)
            nc.gpsimd.dma_start(idm[:], idm_d)
            epsap = singles.tile([P, 1], F32)
            nc.vector.memset(epsap[:], EPS)
            ebias = singles.tile([P, 1], F32)
            nc.vector.memset(ebias[:], EBIAS)

            # ---- GroupNorm stats (channel-major; per channel then 8-chan groups) ----
            alf = small.tile([P, 2], F32, tag="gnalf")   # alpha per ci half
            bet = small.tile([P, 2], F32, tag="gnbet")   # beta per ci half
            for t in range(2):
                st = small.tile([P, 8, 6], F32, tag="gnst")
                xre = xts[t][:, :].rearrange("p (s f) -> p s f", f=512)
                for sg in range(8):
                    nc.vector.bn_stats(st[:, sg, :], xre[:, sg, :])
                mv = small.tile([P, 2], F32, tag="gnmv")
                nc.vector.bn_aggr(mv[:], st[:])
                # stats2 = [mu, E[x^2]] per channel, f32r for the group matmul
                musq = small.tile([P, 1], F32, tag="gnmusq")
                nc.vector.tensor_mul(musq[:], mv[:, 0:1], mv[:, 0:1])
                stats2 = small.tile([P, 2], F32R, tag="gnst2")
                nc.vector.tensor_copy(stats2[:, 0:1], mv[:, 0:1])
                nc.vector.tensor_add(stats2[:, 1:2], mv[:, 1:2], musq[:])
                # group-aggregate (mean over 8 channels)
                gp = pz.tile([P, 2], F32, tag="pz", name=f"gp{t}_")
                nc.tensor.matmul(gp[:], amat[:], stats2[:], start=True, stop=True)
                gs = small.tile([P, 2], F32, tag="gnagg")
                nc.vector.tensor_copy(gs[:], gp[:])
                gvar = small.tile([P, 1], F32, tag="gnvar")
                gmusq = small.tile([P, 1], F32, tag="gnmusq2")
                nc.vector.tensor_mul(gmusq[:], gs[:, 0:1], gs[:, 0:1])
                nc.vector.tensor_tensor(gvar[:], gs[:, 1:2], gmusq[:], ALU.subtract)
                # rstd = exp(-0.5 * ln(var + eps))  (same ACT table set as softmax exp)
                lnv = small.tile([P, 1], F32, tag="gnln")
                nc.scalar.activation(lnv[:], gvar[:], AF.Ln, bias=epsap[:], scale=1.0)
                rstd = small.tile([P, 1], F32, tag="gnrstd")
                nc.scalar.activation(rstd[:], lnv[:], AF.Exp, bias=0.0, scale=-0.5)
                nc.vector.tensor_mul(alf[:, t:t + 1], rstd[:], gnw[:, t:t + 1])
                atmp = small.tile([P, 1], F32, tag="gnatmp")
                nc.vector.tensor_mul(atmp[:], gs[:, 0:1], alf[:, t:t + 1])
                nc.vector.tensor_tensor(bet[:, t:t + 1], gnb[:, t:t + 1], atmp[:],
                                        ALU.subtract)

            # ---- bias plumbing: fold GN beta (and bv, bo) into projection biases ----
            bet_bf = small.tile([P, 2], BF16, tag="betbf")
            nc.vector.tensor_copy(bet_bf[:], bet[:])
            # W @ beta columns for q, k, v (K=1-wide matmuls on raw bf16 weights)
            wbeta = pz.tile([P, 2, 3], F32, tag="pz", name="wbeta")
            for wi, wt in enumerate((wq, wk, wv)):
                for ch in range(2):
                    for ko in range(2):
                        nc.tensor.matmul(wbeta[:, ch, wi:wi + 1],
                                         wt[:, ko, ch * P:(ch + 1) * P],
                                         bet_bf[:, ko:ko + 1],
                                         start=(ko == 0), stop=(ko == 1))
            qbias = small.tile([P, 2], F32, tag="qbias")
            nc.vector.tensor_add(qbias[:], wbeta[:, :, 0], bq[:])
            kbias = small.tile([P, 2], F32, tag="kbias")
            nc.vector.tensor_add(kbias[:], wbeta[:, :, 1], bk[:])
            vbias_bf = small.tile([P, 2], BF16, tag="vbiasbf")
            nc.vector.tensor_add(vbias_bf[:], wbeta[:, :, 2], bvp[:])
            # bfin = (Wo @ (Wv@beta + bv) + bo) * RS2
            obias = pz.tile([P, 2], F32, tag="pz", name="obias")
            for ch in range(2):
                for ko in range(2):
                    nc.tensor.matmul(obias[:, ch:ch + 1],
                                     wo[:, ko, ch * P:(ch + 1) * P],
                                     vbias_bf[:, ko:ko + 1],
                                     start=(ko == 0), stop=(ko == 1))
            # bfinr = Wo@bv'' + bo as bf16 ROWS (partition 0), for the rank-1
            # bias matmul ubias^T @ Z folded into the out-proj psum: after the
            # *RS2/Z normalization that term becomes exactly bfinr*RS2.
            bfinr = small.tile([P, 2], F32, tag="bfinr")
            nc.vector.tensor_add(bfinr[:], obias[:], bop[:])
            bfc = small.tile([P, 2], BF16, tag="bfc")
            nc.vector.tensor_copy(bfc[:], bfinr[:])
            ubp = pz.tile([1, 2, P], F32, tag="pz", name="ubp")
            for ch in range(2):
                nc.tensor.matmul(ubp[0:1, ch, :], bfc[:, ch:ch + 1], idm[:],
                                 start=True, stop=True)
            ubias = small.tile([1, 2, P], BF16, tag="ubias")
            nc.vector.tensor_copy(ubias[:], ubp[:])

            # ---- fold GN alpha into q/k/v weights; cast wo to fp8 pairs ----
            wqf = singles.tile([P, 2, C], BF16, name="wqf")
            wkf = singles.tile([P, 2, C], BF16, name="wkf")
            wvf = singles.tile([P, 2, C], BF16, name="wvf")
            for wt, wf in ((wq, wqf), (wk, wkf), (wv, wvf)):
                for t in range(2):
                    nc.vector.tensor_scalar(wf[:, t, :], wt[:, t, :],
                                            alf[:, t:t + 1], None, ALU.mult)
            wo8 = singles.tile([P, 2, C], F8, name="wo8")
            nc.vector.tensor_copy(wo8[:], wo[:])

            # ---- projections (bf16 weights x bf16 x; outputs cast to fp8) ----
            qT = qk.tile([P, 2, NQ], F8, tag="qT")
            kT = qk.tile([P, 2, N], F8, tag="kT")
            v8 = vpool.tile([P, MT, C], F8)
            # interleave q/k/v blocks so ACT/DVE casts chase the PE
            for blk in range(8):
                # k block: 512 tokens
                kps = ps.tile([P, 2, 512], F32, tag="ps", name=f"kps{blk}")
                for ch in range(2):
                    for ko in range(2):
                        nc.tensor.matmul(kps[:, ch, :],
                                         wkf[:, ko, ch * P:(ch + 1) * P],
                                         xts[ko][:, blk * 512:(blk + 1) * 512],
                                         start=(ko == 0), stop=(ko == 1))
                for ch in range(2):
                    nc.scalar.activation(kT[:, ch, blk * 512:(blk + 1) * 512],
                                         kps[:, ch, :], AF.Identity,
                                         bias=kbias[:, ch:ch + 1], scale=1.0)
                if blk < 4:
                    # q block: 512 tokens (first NQ only)
                    qps = po.tile([P, 2, 512], F32, tag="po", name=f"qps{blk}")
                    for ch in range(2):
                        for ko in range(2):
                            nc.tensor.matmul(qps[:, ch, :],
                                             wqf[:, ko, ch * P:(ch + 1) * P],
                                             xts[ko][:, blk * 512:(blk + 1) * 512],
                                             start=(ko == 0), stop=(ko == 1))
                    for ch in range(2):
                        nc.vector.tensor_scalar(qT[:, ch, blk * 512:(blk + 1) * 512],
                                                qps[:, ch, :], qbias[:, ch:ch + 1],
                                                None, ALU.add)
                # v block: 4 m-tiles (512 tokens)
                vps = ps.tile([P, 2, 512], F32, tag="ps", name=f"vps{blk}")
                vpv = vps[:, :, :].rearrange("p a b -> p (a b)").rearrange(
                    "p (i c) -> p i c", c=C)
                for i in range(4):
                    m = 4 * blk + i
                    for ko in range(2):
                        nc.tensor.matmul(vpv[:, i, :],
                                         xts[ko][:, m * P:(m + 1) * P],
                                         wvf[:, ko, :],
                                         start=(ko == 0), stop=(ko == 1))
                nc.vector.tensor_copy(v8[:, 4 * blk:4 * blk + 4, :], vpv[:])

            # ---- attention strips (fp8 DoubleRow) ----
            NH = NSTRIP // P  # 128-wide query groups per strip
            for s in range(NS):
                ns = slice(s * NSTRIP, (s + 1) * NSTRIP)
                es = espool.tile([P, MT, NSTRIP], F8, tag="es")
                # scores S^T(m-tile) = (k pair).T @ (q pair); exp in 2-m groups
                for g in range(MT // 2):
                    sp = ps.tile([P, 2, NSTRIP], F32, tag="ps", name=f"sp{s}_{g}")
                    for i in range(2):
                        m = 2 * g + i
                        nc.tensor.matmul(sp[:, i, :],
                                         kT[:, 0:2, m * P:(m + 1) * P],
                                         qT[:, 0:2, ns],
                                         start=True, stop=True, perf_mode=DR)
                    nc.scalar.activation(es[:, 2 * g:2 * g + 2, :], sp[:],
                                         AF.Exp, bias=ebias[:], scale=ISCALE)
                # softmax denominators: single DR chain over 16 m-pairs
                zp = pz.tile([1, NSTRIP], F32, tag="pz", name=f"zp{s}")
                for j in range(MT // 2):
                    nc.tensor.matmul(zp[:], ones8[:, 0:2, 0:1],
                                     es[:, 2 * j:2 * j + 2, :],
                                     start=(j == 0), stop=(j == MT // 2 - 1),
                                     perf_mode=DR)
                # attn @ v: DR over m-pairs, accumulated per channel half
                op = po.tile([P, 2, NSTRIP], F32, tag="po", name=f"op{s}")
                for ch in range(2):
                    for j in range(MT // 2):
                        nc.tensor.matmul(op[:, ch, :],
                                         v8[:, 2 * j:2 * j + 2, ch * P:(ch + 1) * P],
                                         es[:, 2 * j:2 * j + 2, :],
                                         start=(j == 0), stop=(j == MT // 2 - 1),
                                         perf_mode=DR)
                # 1/Z: move Z onto partitions (K=1 matmul), reciprocal there,
                # move back (identity matmul), broadcast (RS2 folded into ones2)
                zs = small.tile([1, NSTRIP], BF16, tag="zs")
                nc.vector.tensor_copy(zs[:], zp[:])
                zqt = pz.tile([P, NH], F32, tag="pz", name=f"zqt{s}")
                for h in range(NH):
                    nc.tensor.matmul(zqt[:, h:h + 1], zs[0:1, h * P:(h + 1) * P],
                                     idm[0:1, 0:1], start=True, stop=True)
                rzt = small.tile([P, NH], BF16, tag="rzt")
                with nc.allow_low_precision(reason="bf16 rounding of 1/Z"):
                    nc.vector.reciprocal(rzt[:], zqt[:])
                rzc = pz.tile([1, NSTRIP], F32, tag="pz", name=f"rzc{s}")
                for h in range(NH):
                    nc.tensor.matmul(rzc[0:1, h * P:(h + 1) * P], rzt[:, h:h + 1],
                                     idm[:], start=True, stop=True)
                rzs = small.tile([1, NSTRIP], BF16, tag="rzs")
                nc.vector.tensor_copy(rzs[:], rzc[:])
                rp2 = pz.tile([P, NSTRIP], F32, tag="pz", name=f"rp2{s}")
                for h in range(NH):
                    nc.tensor.matmul(rp2[:, h * P:(h + 1) * P], ones2[:],
                                     rzs[0:1, h * P:(h + 1) * P],
                                     start=True, stop=True)
                rp2s = small.tile([P, NSTRIP], BF16, tag="rp2s")
                nc.vector.tensor_copy(rp2s[:], rp2[:])
                # out projection on unnormalized o (fp8 DR) + rank-1 bias*Z term
                # (becomes the plain bias after the *RS2/Z column scale below)
                o8 = opool.tile([P, 2, NSTRIP], F8, tag="o8")
                nc.vector.tensor_copy(o8[:], op[:])
                op2 = po.tile([P, 2, NSTRIP], F32, tag="po", name=f"op2{s}")
                for ch in range(2):
                    nc.tensor.matmul(op2[:, ch, :], wo8[:, 0:2, ch * P:(ch + 1) * P],
                                     o8[:, 0:2, :], start=True, stop=False,
                                     perf_mode=DR)
                    nc.tensor.matmul(op2[:, ch, :], ubias[0:1, ch, :], zs[:],
                                     start=False, stop=True)
                t1 = zfpool.tile([P, 2, NSTRIP], BF16, tag="t1")
                for ch in range(2):
                    nc.vector.tensor_mul(t1[:, ch, :], op2[:, ch, :], rp2s[:])
                final = zfpool.tile([P, 2, NSTRIP], F32, tag="fin")
                for t in range(2):
                    nc.vector.scalar_tensor_tensor(
                        out=final[:, t, :], in0=xts[t][:, ns], scalar=RS2,
                        in1=t1[:, t, :], op0=ALU.mult, op1=ALU.add)
                    nc.sync.dma_start(out_d[t, :, ns], final[:, t, :])

    nc.finalize()
    return nc


def _get_nc():
    if "nc" not in _prog_cache:
        _prog_cache["nc"] = _build_nc()
    return _prog_cache["nc"]


def _make_in_maps(x, gn_weight, gn_bias, Wq, bq, Wk, bk, Wv, bv, Wo, bo):
    x = np.asarray(x, dtype=np.float32)
    f32 = lambda a: np.ascontiguousarray(np.asarray(a, dtype=np.float32))

    def packT(b_vec):  # [256] -> [128, 2] (c_out_in, c_out_half)
        return np.ascontiguousarray(f32(b_vec).reshape(2, P).T)

    def packW(W):  # [C, C] -> [P, 2, C] bf16 pairs: w[ci_in, ko, co] = W[co, ko*128+ci]
        wT = f32(np.asarray(W).T).reshape(2, P, C)  # [ko, ci_in, co]
        return np.ascontiguousarray(wT.transpose(1, 0, 2)).astype(ml_dtypes.bfloat16)

    amat = np.zeros((P, P), np.float32)
    for g in range(P // GS):
        amat[g * GS:(g + 1) * GS, g * GS:(g + 1) * GS] = 1.0 / GS

    common = {
        "wqT": packW(Wq),
        "wkT": packW(Wk),
        "wvT": packW(Wv),
        "woT": packW(Wo),
        "bqp": packT(bq),
        "bkp": packT(bk),
        "bvp": packT(bv),
        "bop": packT(bo),
        "gnw": packT(gn_weight),
        "gnb": packT(gn_bias),
        "amat": amat,
        "ones2": np.full((1, P), RS2, ml_dtypes.bfloat16),
        "ones8": np.ones((P, 2, 16), ml_dtypes.float8_e4m3),
        "idm": np.eye(P, dtype=ml_dtypes.bfloat16),
    }

    in_maps = []
    for core in range(8):
        b, half = core // 2, core % 2
        xt = x[b].reshape(C, N)
        if half:
            xt = np.roll(xt, -NQ, axis=1)
        xt = np.ascontiguousarray(xt).reshape(2, P, N).astype(ml_dtypes.bfloat16)
        in_maps.append({"xt": xt, **common})
    return in_maps


def _assemble(results, B):
    out = np.empty((B, C, N), np.float32)
    for core in range(2 * B):
        b, half = core // 2, core % 2
        out[b, :, half * NQ:(half + 1) * NQ] = results[core]["out"].reshape(C, NQ)
    return out.reshape(B, C, 64, 64)


def kernel(x, gn_weight, gn_bias, Wq, bq, Wk, bk, Wv, bv, Wo, bo):
    x = np.asarray(x, dtype=np.float32)
    in_maps = _make_in_maps(x, gn_weight, gn_bias, Wq, bq, Wk, bk, Wv, bv, Wo, bo)
    nc = _get_nc()
    res = run_bass_kernel_spmd(nc, in_maps, list(range(8)))
    return _assemble(res.results, x.shape[0])


# revision 36
# speedup vs baseline: 1.8929x; 1.0248x over previous
"""AttnBlock (GroupNorm -> single-head self-attention -> residual) on 8 TRN2 cores.

Sharding: B=4 batch elements x 2 query-token halves = 8 cores (SPMD, no
collectives).  Each core receives the full (rolled) channel-major batch
element x^T [C=256, HW=4096] in bf16, computes GroupNorm stats + k/v for
all 4096 tokens, and q/scores/attention/out-proj for its 2048-token half.
Odd cores get x rolled by -2048 tokens; attention is permutation-invariant
over keys, so their first 2048 tokens are the original tokens 2048:4096.

Matmul strategy: GroupNorm is folded into the projections (alpha into the
bf16 weights, beta into per-channel biases via tiny K=1 matmuls), so the
q/k/v projections read x^T directly.  The attention matmuls (scores,
softmax-denominator chain, attn@v, out-proj) run in fp8e4m3 with
perf_mode=DoubleRow, which packs the full K=256 contraction into a single
PE pass at 2 MACs/cell/cycle.  exp uses a -3 logit bias so the fp8
softmax numerator stays within e4m3 range (the factor cancels in the
normalization).  1/Z is computed after transposing Z onto partitions
(a [1,256] single-lane reciprocal is ~16x slower than a [128,2] one),
and the normalization is applied after the out-projection (a per-query
column scale commutes with the channel-space projection).  bv and bo
fold into one final bias; x + out is scaled by 2^-0.5 at the end.
"""

import numpy as np
import ml_dtypes

import concourse.bass as bass
import concourse.tile as tile
from concourse import bacc, mybir
from concourse.bass_utils import run_bass_kernel_spmd

dt = mybir.dt
F32, F32R, BF16, F8 = dt.float32, dt.float32r, dt.bfloat16, dt.float8e4
AF = mybir.ActivationFunctionType
ALU = mybir.AluOpType
DR = mybir.MatmulPerfMode.DoubleRow

P = 128          # partitions
C = 256          # channels
N = 4096         # tokens per batch element (64*64)
NQ = 2048        # query tokens per core
NSTRIP = 512     # query-token strip width
NS = NQ // NSTRIP  # 8 strips
MT = N // P      # 32 key m-tiles
GS = 8           # channels per group (256 / 32 groups)
EPS = 1e-6
ISCALE = 1.0 / 16.0       # attention scale c**-0.5
EBIAS = -3.0              # exp logit bias; cancels in softmax normalization
RS2 = float(2.0 ** -0.5)  # output residual scale

_prog_cache = {}


def _build_nc():
    nc = bacc.Bacc("TRN2", target_bir_lowering=False, debug=False, num_devices=8)

    def inp(name, shape, d=F32):
        return nc.dram_tensor(name, shape, d, kind="ExternalInput").ap()

    xt_d = inp("xt", [2, P, N], BF16)      # [ci_half, ci_in, n]
    wq_d = inp("wqT", [P, 2, C], BF16)     # [ci_in, ci_half, c_out] = Wq.T pairs
    wk_d = inp("wkT", [P, 2, C], BF16)
    wv_d = inp("wvT", [P, 2, C], BF16)
    wo_d = inp("woT", [P, 2, C], BF16)
    bq_d = inp("bqp", [P, 2])              # [c_out_in, c_out_half]
    bk_d = inp("bkp", [P, 2])
    bv_d = inp("bvp", [P, 2])
    bo_d = inp("bop", [P, 2])
    gnw_d = inp("gnw", [P, 2])
    gnb_d = inp("gnb", [P, 2])
    amat_d = inp("amat", [P, P])           # block-diag 8x8 of 1/8
    ones2_d = inp("ones2", [1, P], BF16)   # value RS2 (folds residual scale into rz)
    ones8_d = inp("ones8", [P, 2, 16], F8)  # 1.0; 16-padded for DR weight AP
    idm_d = inp("idm", [P, P], BF16)       # identity (partition<->free moves via PE)
    out_d = nc.dram_tensor("out", [2, P, NQ], F32, kind="ExternalOutput").ap()

    with tile.TileContext(nc) as tc:
        with (
            tc.tile_pool(name="singles", bufs=1) as singles,
            tc.tile_pool(name="xpool", bufs=1) as xpool,
            tc.tile_pool(name="qk", bufs=1) as qk,
            tc.tile_pool(name="vpool", bufs=1) as vpool,
            tc.tile_pool(name="espool", bufs=2) as espool,
            tc.tile_pool(name="opool", bufs=2) as opool,
            tc.tile_pool(name="small", bufs=2) as small,
            tc.tile_pool(name="zf", bufs=2) as zfpool,
            tc.tile_pool(name="ps", bufs=2, space="PSUM") as ps,      # 2x2 banks
            tc.tile_pool(name="po", bufs=1, space="PSUM") as po,      # 2 banks
            tc.tile_pool(name="pz", bufs=1, space="PSUM") as pz,      # 1 bank
        ):
            # ---- x load first (chunked; bn_stats pipelined behind each chunk) ----
            xt0 = xpool.tile([P, N], BF16, tag="xt0")
            xt1 = xpool.tile([P, N], BF16, tag="xt1")
            xts = (xt0, xt1)
            _dmae = [nc.sync, nc.scalar, nc.gpsimd]
            for t in range(2):
                for h in range(4):
                    _dmae[(4 * t + h) % 3].dma_start(
                        xts[t][:, h * 1024:(h + 1) * 1024],
                        xt_d[t, :, h * 1024:(h + 1) * 1024])

            # ---- weights / constants ----
            wq = singles.tile([P, 2, C], BF16)
            nc.gpsimd.dma_start(wq[:], wq_d)
            wk = singles.tile([P, 2, C], BF16)
            nc.gpsimd.dma_start(wk[:], wk_d)
            wv = singles.tile([P, 2, C], BF16)
            nc.gpsimd.dma_start(wv[:], wv_d)
            wo = singles.tile([P, 2, C], BF16)
            nc.gpsimd.dma_start(wo[:], wo_d)
            bq = singles.tile([P, 2], F32)
            nc.gpsimd.dma_start(bq[:], bq_d)
            bk = singles.tile([P, 2], F32)
            nc.gpsimd.dma_start(bk[:], bk_d)
            bvp = singles.tile([P, 2], F32)
            nc.gpsimd.dma_start(bvp[:], bv_d)
            bop = singles.tile([P, 2], F32)
            nc.gpsimd.dma_start(bop[:], bo_d)
            gnw = singles.tile([P, 2], F32)
            nc.gpsimd.dma_start(gnw[:], gnw_d)
            gnb = singles.tile([P, 2], F32)
            nc.gpsimd.dma_start(gnb[:], gnb_d)
            amat = singles.tile([P, P], F32R)
            nc.gpsimd.dma_start(amat[:], amat_d.bitcast(F32R))
            ones2 = singles.tile([1, P], BF16)
            nc.gpsimd.dma_start(ones2[:], ones2_d)
            ones8 = singles.tile([P, 2, 16], F8)
            nc.gpsimd.dma_start(ones8[:], ones8_d)
            idm = singles.tile([P, P], BF16)
            nc.gpsimd.dma_start(idm[:], idm_d)
            epsap = singles.tile([P, 1], F32)
            nc.vector.memset(epsap[:], EPS)
            ebias = singles.tile([P, 1], F32)
            nc.vector.memset(ebias[:], EBIAS)

            # ---- GroupNorm stats (channel-major; per channel then 8-chan groups;
            # ln/exp batched across both ci halves to avoid ACT table churn) ----
            sts = []
            for t in range(2):
                st = small.tile([P, 8, 6], F32, tag=f"gnst{t}", name=f"gnst{t}")
                xre = xts[t][:, :].rearrange("p (s f) -> p s f", f=512)
                for sg in range(8):
                    nc.vector.bn_stats(st[:, sg, :], xre[:, sg, :])
                sts.append(st)
            gs = small.tile([P, 2, 2], F32, tag="gnagg")  # [ci_in, t, (mu, E[x^2])]
            for t in range(2):
                mv = small.tile([P, 2], F32, tag="gnmv")
                nc.vector.bn_aggr(mv[:], sts[t][:])
                # stats2 = [mu, E[x^2]] per channel, f32r for the group matmul
                musq = small.tile([P, 1], F32, tag="gnmusq")
                nc.vector.tensor_mul(musq[:], mv[:, 0:1], mv[:, 0:1])
                stats2 = small.tile([P, 2], F32R, tag="gnst2")
                nc.vector.tensor_copy(stats2[:, 0:1], mv[:, 0:1])
                nc.vector.tensor_add(stats2[:, 1:2], mv[:, 1:2], musq[:])
                # group-aggregate (mean over 8 channels)
                gp = pz.tile([P, 2], F32, tag="pz", name=f"gp{t}_")
                nc.tensor.matmul(gp[:], amat[:], stats2[:], start=True, stop=True)
                nc.vector.tensor_copy(gs[:, t, :], gp[:])
            gvar = small.tile([P, 2], F32, tag="gnvar")
            gmusq = small.tile([P, 2], F32, tag="gnmusq2")
            nc.vector.tensor_mul(gmusq[:], gs[:, :, 0], gs[:, :, 0])
            nc.vector.tensor_tensor(gvar[:], gs[:, :, 1], gmusq[:], ALU.subtract)
            # rstd = exp(-0.5 * ln(var + eps)), one ln + one exp for both halves
            lnv = small.tile([P, 2], F32, tag="gnln")
            nc.scalar.activation(lnv[:], gvar[:], AF.Ln, bias=epsap[:], scale=1.0)
            rstd = small.tile([P, 2], F32, tag="gnrstd")
            nc.scalar.activation(rstd[:], lnv[:], AF.Exp, bias=0.0, scale=-0.5)
            alf = small.tile([P, 2], F32, tag="gnalf")   # alpha per ci half
            bet = small.tile([P, 2], F32, tag="gnbet")   # beta per ci half
            nc.vector.tensor_mul(alf[:], rstd[:], gnw[:])
            atmp = small.tile([P, 2], F32, tag="gnatmp")
            nc.vector.tensor_mul(atmp[:], gs[:, :, 0], alf[:])
            nc.vector.tensor_tensor(bet[:], gnb[:], atmp[:], ALU.subtract)

            # ---- bias plumbing: fold GN beta (and bv, bo) into projection biases ----
            bet_bf = small.tile([P, 2], BF16, tag="betbf")
            nc.vector.tensor_copy(bet_bf[:], bet[:])
            # W @ beta columns for q, k, v (K=1-wide matmuls on raw bf16 weights)
            wbeta = pz.tile([P, 2, 3], F32, tag="pz", name="wbeta")
            for wi, wt in enumerate((wq, wk, wv)):
                for ch in range(2):
                    for ko in range(2):
                        nc.tensor.matmul(wbeta[:, ch, wi:wi + 1],
                                         wt[:, ko, ch * P:(ch + 1) * P],
                                         bet_bf[:, ko:ko + 1],
                                         start=(ko == 0), stop=(ko == 1))
            qbias = small.tile([P, 2], F32, tag="qbias")
            nc.vector.tensor_add(qbias[:], wbeta[:, :, 0], bq[:])
            kbias = small.tile([P, 2], F32, tag="kbias")
            nc.vector.tensor_add(kbias[:], wbeta[:, :, 1], bk[:])
            vbias_bf = small.tile([P, 2], BF16, tag="vbiasbf")
            nc.vector.tensor_add(vbias_bf[:], wbeta[:, :, 2], bvp[:])
            # bfin = (Wo @ (Wv@beta + bv) + bo) * RS2
            obias = pz.tile([P, 2], F32, tag="pz", name="obias")
            for ch in range(2):
                for ko in range(2):
                    nc.tensor.matmul(obias[:, ch:ch + 1],
                                     wo[:, ko, ch * P:(ch + 1) * P],
                                     vbias_bf[:, ko:ko + 1],
                                     start=(ko == 0), stop=(ko == 1))
            # bfinr = Wo@bv'' + bo as bf16 ROWS (partition 0), for the rank-1
            # bias matmul ubias^T @ Z folded into the out-proj psum: after the
            # *RS2/Z normalization that term becomes exactly bfinr*RS2.
            bfinr = small.tile([P, 2], F32, tag="bfinr")
            nc.vector.tensor_add(bfinr[:], obias[:], bop[:])
            bfc = small.tile([P, 2], BF16, tag="bfc")
            nc.vector.tensor_copy(bfc[:], bfinr[:])
            ubp = pz.tile([1, 2, P], F32, tag="pz", name="ubp")
            for ch in range(2):
                nc.tensor.matmul(ubp[0:1, ch, :], bfc[:, ch:ch + 1], idm[:],
                                 start=True, stop=True)
            ubias = small.tile([1, 2, P], BF16, tag="ubias")
            nc.vector.tensor_copy(ubias[:], ubp[:])

            # ---- fold GN alpha into q/k/v weights; cast wo to fp8 pairs ----
            wqf = singles.tile([P, 2, C], BF16, name="wqf")
            wkf = singles.tile([P, 2, C], BF16, name="wkf")
            wvf = singles.tile([P, 2, C], BF16, name="wvf")
            for wt, wf in ((wq, wqf), (wk, wkf), (wv, wvf)):
                for t in range(2):
                    nc.vector.tensor_scalar(wf[:, t, :], wt[:, t, :],
                                            alf[:, t:t + 1], None, ALU.mult)
            wo8 = singles.tile([P, 2, C], F8, name="wo8")
            nc.vector.tensor_copy(wo8[:], wo[:])

            # ---- projections (bf16 weights x bf16 x; outputs cast to fp8) ----
            qT = qk.tile([P, 2, NQ], F8, tag="qT")
            kT = qk.tile([P, 2, N], F8, tag="kT")
            v8 = vpool.tile([P, MT, C], F8)
            # interleave q/k/v blocks so ACT/DVE casts chase the PE
            for blk in range(8):
                # k block: 512 tokens
                kps = ps.tile([P, 2, 512], F32, tag="ps", name=f"kps{blk}")
                for ch in range(2):
                    for ko in range(2):
                        nc.tensor.matmul(kps[:, ch, :],
                                         wkf[:, ko, ch * P:(ch + 1) * P],
                                         xts[ko][:, blk * 512:(blk + 1) * 512],
                                         start=(ko == 0), stop=(ko == 1))
                for ch in range(2):
                    nc.scalar.activation(kT[:, ch, blk * 512:(blk + 1) * 512],
                                         kps[:, ch, :], AF.Identity,
                                         bias=kbias[:, ch:ch + 1], scale=1.0)
                if blk < 4:
                    # q block: 512 tokens (first NQ only)
                    qps = po.tile([P, 2, 512], F32, tag="po", name=f"qps{blk}")
                    for ch in range(2):
                        for ko in range(2):
                            nc.tensor.matmul(qps[:, ch, :],
                                             wqf[:, ko, ch * P:(ch + 1) * P],
                                             xts[ko][:, blk * 512:(blk + 1) * 512],
                                             start=(ko == 0), stop=(ko == 1))
                    for ch in range(2):
                        nc.vector.tensor_scalar(qT[:, ch, blk * 512:(blk + 1) * 512],
                                                qps[:, ch, :], qbias[:, ch:ch + 1],
                                                None, ALU.add)
                # v block: 4 m-tiles (512 tokens)
                vps = ps.tile([P, 2, 512], F32, tag="ps", name=f"vps{blk}")
                vpv = vps[:, :, :].rearrange("p a b -> p (a b)").rearrange(
                    "p (i c) -> p i c", c=C)
                for i in range(4):
                    m = 4 * blk + i
                    for ko in range(2):
                        nc.tensor.matmul(vpv[:, i, :],
                                         xts[ko][:, m * P:(m + 1) * P],
                                         wvf[:, ko, :],
                                         start=(ko == 0), stop=(ko == 1))
                nc.vector.tensor_copy(v8[:, 4 * blk:4 * blk + 4, :], vpv[:])

            # ---- attention strips (fp8 DoubleRow); last strips narrower so the
            # final exposed epilogue chain is short ----
            strips = [(0, 512), (512, 512), (1024, 512), (1536, 256), (1792, 256)]
            for s, (off, w) in enumerate(strips):
                ns = slice(off, off + w)
                es = espool.tile([P, MT, w], F8, tag="es", name=f"es{s}")
                # scores S^T(m-tile) = (k pair).T @ (q pair); exp in 2-m groups
                for g in range(MT // 2):
                    sp = ps.tile([P, 2, w], F32, tag="ps", name=f"sp{s}_{g}")
                    for i in range(2):
                        m = 2 * g + i
                        nc.tensor.matmul(sp[:, i, :],
                                         kT[:, 0:2, m * P:(m + 1) * P],
                                         qT[:, 0:2, ns],
                                         start=True, stop=True, perf_mode=DR)
                    nc.scalar.activation(es[:, 2 * g:2 * g + 2, :], sp[:],
                                         AF.Exp, bias=ebias[:], scale=ISCALE)
                # softmax denominators: single DR chain over 16 m-pairs
                zp = pz.tile([1, w], F32, tag="pz", name=f"zp{s}")
                for j in range(MT // 2):
                    nc.tensor.matmul(zp[:], ones8[:, 0:2, 0:1],
                                     es[:, 2 * j:2 * j + 2, :],
                                     start=(j == 0), stop=(j == MT // 2 - 1),
                                     perf_mode=DR)
                # 1/Z: move Z onto partitions (K=1 matmul), reciprocal there,
                # move back (identity matmul), broadcast (RS2 folded into ones2)
                zs = small.tile([1, w], BF16, tag="zs", name=f"zs{s}")
                nc.vector.tensor_copy(zs[:], zp[:])
                zqt = pz.tile([P, w // P], F32, tag="pz", name=f"zqt{s}")
                for h in range(w // P):
                    nc.tensor.matmul(zqt[:, h:h + 1], zs[0:1, h * P:(h + 1) * P],
                                     idm[0:1, 0:1], start=True, stop=True)
                rzt = small.tile([P, w // P], BF16, tag="rzt", name=f"rzt{s}")
                with nc.allow_low_precision(reason="bf16 rounding of 1/Z"):
                    nc.vector.reciprocal(rzt[:], zqt[:])
                rzc = pz.tile([1, w], F32, tag="pz", name=f"rzc{s}")
                for h in range(w // P):
                    nc.tensor.matmul(rzc[0:1, h * P:(h + 1) * P], rzt[:, h:h + 1],
                                     idm[:], start=True, stop=True)
                rzs = small.tile([1, w], BF16, tag="rzs", name=f"rzs{s}")
                nc.vector.tensor_copy(rzs[:], rzc[:])
                rp2 = pz.tile([P, w], F32, tag="pz", name=f"rp2{s}")
                for h in range(w // P):
                    nc.tensor.matmul(rp2[:, h * P:(h + 1) * P], ones2[:],
                                     rzs[0:1, h * P:(h + 1) * P],
                                     start=True, stop=True)
                rp2s = small.tile([P, w], BF16, tag="rp2s", name=f"rp2s{s}")
                nc.vector.tensor_copy(rp2s[:], rp2[:])
                # attn @ v: DR over m-pairs, accumulated per channel half
                op = po.tile([P, 2, w], F32, tag="po", name=f"op{s}")
                for ch in range(2):
                    for j in range(MT // 2):
                        nc.tensor.matmul(op[:, ch, :],
                                         v8[:, 2 * j:2 * j + 2, ch * P:(ch + 1) * P],
                                         es[:, 2 * j:2 * j + 2, :],
                                         start=(j == 0), stop=(j == MT // 2 - 1),
                                         perf_mode=DR)
                # out projection on unnormalized o (fp8 DR) + rank-1 bias*Z term
                # (becomes the plain bias after the *RS2/Z column scale below)
                o8 = opool.tile([P, 2, w], F8, tag="o8", name=f"o8{s}")
                nc.vector.tensor_copy(o8[:], op[:])
                op2 = po.tile([P, 2, w], F32, tag="po", name=f"op2{s}")
                for ch in range(2):
                    nc.tensor.matmul(op2[:, ch, :], wo8[:, 0:2, ch * P:(ch + 1) * P],
                                     o8[:, 0:2, :], start=True, stop=False,
                                     perf_mode=DR)
                    nc.tensor.matmul(op2[:, ch, :], ubias[0:1, ch, :], zs[:],
                                     start=False, stop=True)
                t1 = zfpool.tile([P, 2, w], BF16, tag="t1", name=f"t1{s}")
                for ch in range(2):
                    nc.vector.tensor_mul(t1[:, ch, :], op2[:, ch, :], rp2s[:])
                final = zfpool.tile([P, 2, w], F32, tag="fin", name=f"fin{s}")
                for t in range(2):
                    nc.vector.scalar_tensor_tensor(
                        out=final[:, t, :], in0=xts[t][:, ns], scalar=RS2,
                        in1=t1[:, t, :], op0=ALU.mult, op1=ALU.add)
                    nc.sync.dma_start(out_d[t, :, ns], final[:, t, :])

    nc.finalize()
    return nc


def _get_nc():
    if "nc" not in _prog_cache:
        _prog_cache["nc"] = _build_nc()
    return _prog_cache["nc"]


def _make_in_maps(x, gn_weight, gn_bias, Wq, bq, Wk, bk, Wv, bv, Wo, bo):
    x = np.asarray(x, dtype=np.float32)
    f32 = lambda a: np.ascontiguousarray(np.asarray(a, dtype=np.float32))

    def packT(b_vec):  # [256] -> [128, 2] (c_out_in, c_out_half)
        return np.ascontiguousarray(f32(b_vec).reshape(2, P).T)

    def packW(W):  # [C, C] -> [P, 2, C] bf16 pairs: w[ci_in, ko, co] = W[co, ko*128+ci]
        wT = f32(np.asarray(W).T).reshape(2, P, C)  # [ko, ci_in, co]
        return np.ascontiguousarray(wT.transpose(1, 0, 2)).astype(ml_dtypes.bfloat16)

    amat = np.zeros((P, P), np.float32)
    for g in range(P // GS):
        amat[g * GS:(g + 1) * GS, g * GS:(g + 1) * GS] = 1.0 / GS

    common = {
        "wqT": packW(Wq),
        "wkT": packW(Wk),
        "wvT": packW(Wv),
        "woT": packW(Wo),
        "bqp": packT(bq),
        "bkp": packT(bk),
        "bvp": packT(bv),
        "bop": packT(bo),
        "gnw": packT(gn_weight),
        "gnb": packT(gn_bias),
        "amat": amat,
        "ones2": np.full((1, P), RS2, ml_dtypes.bfloat16),
        "ones8": np.ones((P, 2, 16), ml_dtypes.float8_e4m3),
        "idm": np.eye(P, dtype=ml_dtypes.bfloat16),
    }

    in_maps = []
    for core in range(8):
        b, half = core // 2, core % 2
        xt = x[b].reshape(C, N)
        if half:
            xt = np.roll(xt, -NQ, axis=1)
        xt = np.ascontiguousarray(xt).reshape(2, P, N).astype(ml_dtypes.bfloat16)
        in_maps.append({"xt": xt, **common})
    return in_maps


def _assemble(results, B):
    out = np.empty((B, C, N), np.float32)
    for core in range(2 * B):
        b, half = core // 2, core % 2
        out[b, :, half * NQ:(half + 1) * NQ] = results[core]["out"].reshape(C, NQ)
    return out.reshape(B, C, 64, 64)


def kernel(x, gn_weight, gn_bias, Wq, bq, Wk, bk, Wv, bv, Wo, bo):
    x = np.asarray(x, dtype=np.float32)
    in_maps = _make_in_maps(x, gn_weight, gn_bias, Wq, bq, Wk, bk, Wv, bv, Wo, bo)
    nc = _get_nc()
    res = run_bass_kernel_spmd(nc, in_maps, list(range(8)))
    return _assemble(res.results, x.shape[0])
